# revision 19
# baseline (speedup 1.0000x reference)
"""Binarized 3x3 conv (BinarizeConv2dSDP) for one TRN2 chip (8 NeuronCores).

Reference computation:
    out = conv2d(sign(x), sign(M), stride=1, pad=1) * Alpha      (all fp32)
    x: (32, 256, 56, 56)   M: (256, 256, 3, 3)   Alpha: (256, 1, 1)

Strategy (per the data-parallel sharding hint):
  - Shard x over batch: 4 images per core; replicate M/Alpha on every core.
  - On-core: binarize x and M to fp8 (+/-1 exactly representable), run the
    conv as 9 shifted DoubleRow matmuls (contraction = 256 channels in one
    pass: 128 partitions x 2 pair-rows) accumulating in PSUM, scale by
    Alpha while evacuating PSUM, DMA out fp32.
  - Activations live in SBUF as zero-padded 58x58 images so every (kh,kw)
    tap of the 3x3 kernel is just a flat column offset; one matmul computes
    an 8-output-row strip (8*58 = 464 psum columns, garbage columns at the
    row seams are simply not copied out).
"""

import numpy as np

import concourse.bacc as bacc
import concourse.bass as bass
import concourse.tile as tile
from concourse import masks, mybir
from concourse.bass_utils import run_bass_kernel_spmd

F32 = mybir.dt.float32
BF16 = mybir.dt.bfloat16
FP8 = mybir.dt.float8e4

# ---- problem geometry (hardcoded; kernel.py must be self-contained) ----
N_CORES = 8
NB = 4          # images per core (32 / 8)
C = 256         # in channels  (2 halves of 128 partitions)
O = 256         # out channels (2 tiles of 128 partitions)
H = W = 56
K = 3
PW = H + 2      # padded row width  (58)
NPIX = PW * PW  # padded image size (3364)
PH = 3376       # padded image allocation (multiple of 16 for DoubleRow step)
RS = 8          # output rows per strip
NSTRIP = H // RS        # 7
NCOL = RS * PW          # 464 psum columns per strip (<= 512 fp32 bank)
NVAL = RS * W           # 448 valid columns per strip


def build_nc(paired: bool = True) -> bass.Bass:
    """Build the SPMD Bass program for one core's shard.

    paired=True : fp8 DoubleRow matmuls (K=256 per instruction, 9 per strip)
    paired=False: plain matmuls (K=128, 18 per strip) - debug fallback
    """
    act_dt = FP8 if paired else BF16
    nc = bacc.Bacc("TRN2")

    x = nc.declare_dram_parameter("x", [NB, C, H, W], F32, isOutput=False)
    m = nc.declare_dram_parameter("m", [O, C, K, K], F32, isOutput=False)
    alpha = nc.declare_dram_parameter("alpha", [O], F32, isOutput=False)
    out = nc.declare_dram_parameter("out", [NB, O, H, W], F32, isOutput=True)

    with tile.TileContext(nc) as tc:
        with (
            tc.tile_pool(name="consts", bufs=1) as consts,
            tc.tile_pool(name="xsrc", bufs=6) as xsrc_pool,
            tc.tile_pool(name="wsrc", bufs=4) as wsrc_pool,
            tc.tile_pool(name="wsgn", bufs=4) as wsgn_pool,
            tc.tile_pool(name="osb", bufs=4) as osb_pool,
            tc.tile_pool(name="warm", bufs=1, space="PSUM") as warm_pool,
            tc.tile_pool(name="ptr", bufs=2, space="PSUM") as ptr_pool,
            tc.tile_pool(name="pmm", bufs=5, space="PSUM") as pmm_pool,
        ):
            # ---- weights: sign -> transpose(c<->o) -> packed lhsT ----
            # One (ot, half) unit = one 590KB DMA + one sign + 9 transposes,
            # so the first transposes can start as soon as the first quarter
            # of M has landed. w DMAs are issued first on the sync queue.
            # wbuf[c2, half, k*256 + ot*128 + o] = sign(M[ot*128+o, half*128+c2, kh, kw])
            wbuf = consts.tile([128, 2, K * K * O], act_dt)
            wprep = []
            for ot in range(2):
                for half in range(2):
                    wsrc = wsrc_pool.tile([128, 128 * K * K], F32)
                    nc.sync.dma_start(
                        out=wsrc[:],
                        in_=m[
                            ot * 128 : (ot + 1) * 128, half * 128 : (half + 1) * 128
                        ].rearrange("o c kh kw -> o (c kh kw)"),
                    )
                    wprep.append((ot, half, wsrc))

            # activation buffer: zero the borders first (no dependencies,
            # runs during the prologue while DVE is otherwise idle)
            act = consts.tile([128, 2 * NB, PH], act_dt)
            for n in range(NB):
                nc.vector.memset(
                    act[:, 2 * n : 2 * n + 2, :]
                    .rearrange("p a b -> p (a b)")
                    .bitcast(mybir.dt.uint32),
                    0,
                )

            # ---- PE warm-up: dependency-free matmuls so the HAM clock gate
            # reaches 2.4 GHz before the real matmuls start ----
            wz = consts.tile([128, 128], BF16)
            nc.vector.memset(wz[:], 0)
            pwarm = warm_pool.tile([128, 128], F32)
            for _ in range(72):
                nc.tensor.matmul(pwarm[:], wz[:], wz[:], start=True, stop=True)

            # ---- constants ----
            identity = consts.tile([128, 128], BF16)
            masks.make_identity(nc, identity[:])

            for ot, half, wsrc in wprep:
                wsgn = wsgn_pool.tile([128, 128 * K * K], BF16)
                nc.scalar.sign(wsgn[:], wsrc[:])
                wsgn_ck = wsgn.rearrange("o (c k) -> o c k", k=K * K)
                for kk in range(K * K):
                    tp = ptr_pool.tile([128, 128], BF16)
                    nc.tensor.transpose(tp[:], wsgn_ck[:, :, kk], identity[:])
                    nc.vector.tensor_copy(
                        wbuf[:, half, kk * O + ot * 128 : kk * O + ot * 128 + 128],
                        tp[:],
                    )

            # ---- activations: zero-padded, binarized, fp8 ----
            # act[c2, 2*n + half, ph*58 + pw] = sign(x[n, half*128+c2, ph-1, pw-1])
            # One DMA per (n, half): DMA triggers serialize at ~600ns each on
            # the sync queue, so fewer/bigger transfers start flowing sooner.
            deferred = []
            for n in range(NB):
                for half in range(2):
                    xs = xsrc_pool.tile([128, H * W], F32)
                    nc.sync.dma_start(
                        out=xs[:],
                        in_=x[n, half * 128 : (half + 1) * 128].rearrange(
                            "c h w -> c (h w)"
                        ),
                    )
                    dst = act[:, 2 * n + half, : PW * PW].rearrange(
                        "p (h w) -> p h w", w=PW
                    )[:, 1 : H + 1, 1 : W + 1]
                    src = xs.rearrange("p (h w) -> p h w", w=W)
                    if n < 2:
                        # ACT engine: sign(x) -> +/-1
                        nc.scalar.sign(dst, src)
                    else:
                        # DVE: (x >= 0) - 0.5 -> +/-0.5 (exact in fp8),
                        # compensated by 2x in this image's output scale.
                        # Emission is deferred into the main loop so these
                        # queue AFTER the first evacuations on the DVE FIFO
                        # and never block PSUM drainage.
                        def _emit(dst=dst, src=src):
                            nc.vector.tensor_scalar(
                                dst,
                                src,
                                0.0,
                                0.5,
                                mybir.AluOpType.is_ge,
                                mybir.AluOpType.subtract,
                            )

                        deferred.append(_emit)

            # alpha is only needed by the first evacuation (~30us in); issue
            # its (slow, scattered) triggers after the x loads
            alpha_sb = consts.tile([128, 2], F32)
            for ot in range(2):
                nc.sync.dma_start(
                    out=alpha_sb[:, ot : ot + 1],
                    in_=alpha.rearrange("(t o) -> t o", t=2)[ot].rearrange(
                        "(o u) -> o u", u=1
                    ),
                )

            # ---- main conv loop ----
            for n in range(NB):
                for s in range(NSTRIP):
                    for ot in range(2):
                        pm = pmm_pool.tile([128, NCOL], F32)
                        mm = 0
                        for kk in range(K * K):
                            kh, kw = divmod(kk, K)
                            base = (RS * s + kh) * PW + kw
                            lhsT2 = wbuf[:, :, kk * O + ot * 128 : kk * O + ot * 128 + 128]
                            rhs2 = act[:, 2 * n : 2 * n + 2, base : base + NCOL]
                            if paired:
                                nc.tensor.matmul(
                                    pm[:],
                                    lhsT2,
                                    rhs2,
                                    start=(mm == 0),
                                    stop=(kk == K * K - 1),
                                    perf_mode=mybir.MatmulPerfMode.DoubleRow,
                                )
                                mm += 1
                            else:
                                for half in range(2):
                                    nc.tensor.matmul(
                                        pm[:],
                                        lhsT2[:, half, :],
                                        rhs2[:, half, :],
                                        start=(mm == 0),
                                        stop=(kk == K * K - 1 and half == 1),
                                    )
                                    mm += 1
                        # evacuate valid columns, scaled by per-channel alpha
                        # (2x extra for images binarized to +/-0.5 on DVE)
                        osb = osb_pool.tile([128, NVAL], F32)
                        if n < 2:
                            nc.vector.tensor_scalar_mul(
                                osb.rearrange("p (r w) -> p r w", w=W),
                                pm.rearrange("p (r w) -> p r w", w=PW)[:, :, :W],
                                alpha_sb[:, ot : ot + 1],
                            )
                        else:
                            nc.vector.tensor_scalar(
                                osb.rearrange("p (r w) -> p r w", w=W),
                                pm.rearrange("p (r w) -> p r w", w=PW)[:, :, :W],
                                alpha_sb[:, ot : ot + 1],
                                2.0,
                                mybir.AluOpType.mult,
                                mybir.AluOpType.mult,
                            )
                        nc.sync.dma_start(
                            out=out[
                                n, ot * 128 : (ot + 1) * 128, RS * s : RS * (s + 1), :
                            ].rearrange("o h w -> o (h w)"),
                            in_=osb[:],
                        )
                        if deferred:
                            deferred.pop(0)()
    nc.finalize()
    return nc


_NC_CACHE: dict[bool, bass.Bass] = {}


def get_nc(paired: bool = True) -> bass.Bass:
    if paired not in _NC_CACHE:
        _NC_CACHE[paired] = build_nc(paired)
    return _NC_CACHE[paired]


def kernel(x: np.ndarray, M: np.ndarray, Alpha: np.ndarray) -> np.ndarray:
    """Full (unsharded) inputs in, full output out. Runs on 8 NeuronCores."""
    assert x.shape == (N_CORES * NB, C, H, W), x.shape
    nc = get_nc(paired=True)
    x = np.ascontiguousarray(x, dtype=np.float32)
    M = np.ascontiguousarray(M, dtype=np.float32)
    a = np.ascontiguousarray(Alpha, dtype=np.float32).reshape(O)
    in_maps = [
        {"x": x[i * NB : (i + 1) * NB], "m": M, "alpha": a} for i in range(N_CORES)
    ]
    res = run_bass_kernel_spmd(nc, in_maps, list(range(N_CORES)))
    return np.concatenate([res.results[i]["out"] for i in range(N_CORES)], axis=0)


# revision 20
# speedup vs baseline: 1.0156x; 1.0156x over previous
"""Binarized 3x3 conv (BinarizeConv2dSDP) for one TRN2 chip (8 NeuronCores).

Reference computation:
    out = conv2d(sign(x), sign(M), stride=1, pad=1) * Alpha      (all fp32)
    x: (32, 256, 56, 56)   M: (256, 256, 3, 3)   Alpha: (256, 1, 1)

Strategy (per the data-parallel sharding hint):
  - Shard x over batch: 4 images per core; replicate M/Alpha on every core.
  - On-core: binarize x and M to fp8 (+/-1 exactly representable), run the
    conv as 9 shifted DoubleRow matmuls (contraction = 256 channels in one
    pass: 128 partitions x 2 pair-rows) accumulating in PSUM, scale by
    Alpha while evacuating PSUM, DMA out fp32.
  - Activations live in SBUF as zero-padded 58x58 images so every (kh,kw)
    tap of the 3x3 kernel is just a flat column offset; one matmul computes
    an 8-output-row strip (8*58 = 464 psum columns, garbage columns at the
    row seams are simply not copied out).
"""

import numpy as np

import concourse.bacc as bacc
import concourse.bass as bass
import concourse.tile as tile
from concourse import masks, mybir
from concourse.bass_utils import run_bass_kernel_spmd

F32 = mybir.dt.float32
BF16 = mybir.dt.bfloat16
FP8 = mybir.dt.float8e4

# ---- problem geometry (hardcoded; kernel.py must be self-contained) ----
N_CORES = 8
NB = 4          # images per core (32 / 8)
C = 256         # in channels  (2 halves of 128 partitions)
O = 256         # out channels (2 tiles of 128 partitions)
H = W = 56
K = 3
PW = H + 2      # padded row width  (58)
NPIX = PW * PW  # padded image size (3364)
PH = 3376       # padded image allocation (multiple of 16 for DoubleRow step)
RS = 8          # output rows per strip
NSTRIP = H // RS        # 7
NCOL = RS * PW          # 464 psum columns per strip (<= 512 fp32 bank)
NVAL = RS * W           # 448 valid columns per strip


def build_nc(paired: bool = True) -> bass.Bass:
    """Build the SPMD Bass program for one core's shard.

    paired=True : fp8 DoubleRow matmuls (K=256 per instruction, 9 per strip)
    paired=False: plain matmuls (K=128, 18 per strip) - debug fallback
    """
    act_dt = FP8 if paired else BF16
    nc = bacc.Bacc("TRN2")

    x = nc.declare_dram_parameter("x", [NB, C, H, W], F32, isOutput=False)
    m = nc.declare_dram_parameter("m", [O, C, K, K], F32, isOutput=False)
    alpha = nc.declare_dram_parameter("alpha", [O], F32, isOutput=False)
    out = nc.declare_dram_parameter("out", [NB, O, H, W], F32, isOutput=True)

    with tile.TileContext(nc) as tc:
        with (
            tc.tile_pool(name="consts", bufs=1) as consts,
            tc.tile_pool(name="xsrc", bufs=4) as xsrc_pool,
            tc.tile_pool(name="wsrc", bufs=4) as wsrc_pool,
            tc.tile_pool(name="wsgn", bufs=4) as wsgn_pool,
            tc.tile_pool(name="osb", bufs=4) as osb_pool,
            tc.tile_pool(name="warm", bufs=1, space="PSUM") as warm_pool,
            tc.tile_pool(name="ptr", bufs=2, space="PSUM") as ptr_pool,
            tc.tile_pool(name="pmm", bufs=5, space="PSUM") as pmm_pool,
        ):
            # ---- weights: sign -> transpose(c<->o) -> packed lhsT ----
            # One (ot, half) unit = one 590KB DMA + one sign + 9 transposes,
            # so the first transposes can start as soon as the first quarter
            # of M has landed. w DMAs are issued first on the sync queue.
            # wbuf[c2, half, k*256 + ot*128 + o] = sign(M[ot*128+o, half*128+c2, kh, kw])
            # alpha: two tiny scattered DMAs; trigger them first so they get
            # queue credit before the big transfers monopolize the DGE ring
            alpha_sb = consts.tile([128, 2], F32)
            for ot in range(2):
                nc.sync.dma_start(
                    out=alpha_sb[:, ot : ot + 1],
                    in_=alpha.rearrange("(t o) -> t o", t=2)[ot].rearrange(
                        "(o u) -> o u", u=1
                    ),
                )

            wbuf = consts.tile([128, 2, K * K * O], act_dt)
            wprep = []
            for ot in range(2):
                for half in range(2):
                    wsrc = wsrc_pool.tile([128, 128 * K * K], F32)
                    nc.sync.dma_start(
                        out=wsrc[:],
                        in_=m[
                            ot * 128 : (ot + 1) * 128, half * 128 : (half + 1) * 128
                        ].rearrange("o c kh kw -> o (c kh kw)"),
                    )
                    wprep.append((ot, half, wsrc))

            # ---- PE warm-up: dependency-free matmuls so the HAM clock gate
            # reaches 2.4 GHz before the real matmuls start ----
            wz = consts.tile([128, 128], BF16)
            nc.vector.memset(wz[:], 0)
            pwarm = warm_pool.tile([128, 128], F32)
            for _ in range(72):
                nc.tensor.matmul(pwarm[:], wz[:], wz[:], start=True, stop=True)

            # activation buffer: zero the borders first (no dependencies,
            # runs during the prologue while DVE is otherwise idle)
            act = consts.tile([128, 2 * NB, PH], act_dt)
            for n in range(NB):
                nc.vector.memset(
                    act[:, 2 * n : 2 * n + 2, :]
                    .rearrange("p a b -> p (a b)")
                    .bitcast(mybir.dt.uint32),
                    0,
                )

            # ---- constants ----
            identity = consts.tile([128, 128], BF16)
            masks.make_identity(nc, identity[:])

            for ot, half, wsrc in wprep:
                wsgn = wsgn_pool.tile([128, 128 * K * K], BF16)
                nc.scalar.sign(wsgn[:], wsrc[:])
                wsgn_ck = wsgn.rearrange("o (c k) -> o c k", k=K * K)
                for kk in range(K * K):
                    tp = ptr_pool.tile([128, 128], BF16)
                    nc.tensor.transpose(tp[:], wsgn_ck[:, :, kk], identity[:])
                    nc.vector.tensor_copy(
                        wbuf[:, half, kk * O + ot * 128 : kk * O + ot * 128 + 128],
                        tp[:],
                    )

            # ---- activations: zero-padded, binarized, fp8 ----
            # act[c2, 2*n + half, ph*58 + pw] = sign(x[n, half*128+c2, ph-1, pw-1])
            # One DMA per (n, half): DMA triggers serialize at ~600ns each on
            # the sync queue, so fewer/bigger transfers start flowing sooner.
            deferred = []
            for n in range(NB):
                for half in range(2):
                    xs = xsrc_pool.tile([128, H * W], F32)
                    nc.sync.dma_start(
                        out=xs[:],
                        in_=x[n, half * 128 : (half + 1) * 128].rearrange(
                            "c h w -> c (h w)"
                        ),
                    )
                    dst = act[:, 2 * n + half, : PW * PW].rearrange(
                        "p (h w) -> p h w", w=PW
                    )[:, 1 : H + 1, 1 : W + 1]
                    src = xs.rearrange("p (h w) -> p h w", w=W)
                    if n < 2:
                        # ACT engine: sign(x) -> +/-1
                        nc.scalar.sign(dst, src)
                    else:
                        # DVE: (x >= 0) - 0.5 -> +/-0.5 (exact in fp8),
                        # compensated by 2x in this image's output scale.
                        # Emission is deferred into the main loop so these
                        # queue AFTER the first evacuations on the DVE FIFO
                        # and never block PSUM drainage.
                        def _emit(dst=dst, src=src):
                            nc.vector.tensor_scalar(
                                dst,
                                src,
                                0.0,
                                0.5,
                                mybir.AluOpType.is_ge,
                                mybir.AluOpType.subtract,
                            )

                        deferred.append(_emit)

            # ---- main conv loop ----
            for n in range(NB):
                for s in range(NSTRIP):
                    for ot in range(2):
                        pm = pmm_pool.tile([128, NCOL], F32)
                        mm = 0
                        for kk in range(K * K):
                            kh, kw = divmod(kk, K)
                            base = (RS * s + kh) * PW + kw
                            lhsT2 = wbuf[:, :, kk * O + ot * 128 : kk * O + ot * 128 + 128]
                            rhs2 = act[:, 2 * n : 2 * n + 2, base : base + NCOL]
                            if paired:
                                nc.tensor.matmul(
                                    pm[:],
                                    lhsT2,
                                    rhs2,
                                    start=(mm == 0),
                                    stop=(kk == K * K - 1),
                                    perf_mode=mybir.MatmulPerfMode.DoubleRow,
                                )
                                mm += 1
                            else:
                                for half in range(2):
                                    nc.tensor.matmul(
                                        pm[:],
                                        lhsT2[:, half, :],
                                        rhs2[:, half, :],
                                        start=(mm == 0),
                                        stop=(kk == K * K - 1 and half == 1),
                                    )
                                    mm += 1
                        # evacuate valid columns, scaled by per-channel alpha
                        # (2x extra for images binarized to +/-0.5 on DVE)
                        osb = osb_pool.tile([128, NVAL], F32)
                        if n < 2:
                            nc.vector.tensor_scalar_mul(
                                osb.rearrange("p (r w) -> p r w", w=W),
                                pm.rearrange("p (r w) -> p r w", w=PW)[:, :, :W],
                                alpha_sb[:, ot : ot + 1],
                            )
                        else:
                            nc.vector.tensor_scalar(
                                osb.rearrange("p (r w) -> p r w", w=W),
                                pm.rearrange("p (r w) -> p r w", w=PW)[:, :, :W],
                                alpha_sb[:, ot : ot + 1],
                                2.0,
                                mybir.AluOpType.mult,
                                mybir.AluOpType.mult,
                            )
                        nc.sync.dma_start(
                            out=out[
                                n, ot * 128 : (ot + 1) * 128, RS * s : RS * (s + 1), :
                            ].rearrange("o h w -> o (h w)"),
                            in_=osb[:],
                        )
                        if deferred:
                            deferred.pop(0)()
    nc.finalize()
    return nc


_NC_CACHE: dict[bool, bass.Bass] = {}


def get_nc(paired: bool = True) -> bass.Bass:
    if paired not in _NC_CACHE:
        _NC_CACHE[paired] = build_nc(paired)
    return _NC_CACHE[paired]


def kernel(x: np.ndarray, M: np.ndarray, Alpha: np.ndarray) -> np.ndarray:
    """Full (unsharded) inputs in, full output out. Runs on 8 NeuronCores."""
    assert x.shape == (N_CORES * NB, C, H, W), x.shape
    nc = get_nc(paired=True)
    x = np.ascontiguousarray(x, dtype=np.float32)
    M = np.ascontiguousarray(M, dtype=np.float32)
    a = np.ascontiguousarray(Alpha, dtype=np.float32).reshape(O)
    in_maps = [
        {"x": x[i * NB : (i + 1) * NB], "m": M, "alpha": a} for i in range(N_CORES)
    ]
    res = run_bass_kernel_spmd(nc, in_maps, list(range(N_CORES)))
    return np.concatenate([res.results[i]["out"] for i in range(N_CORES)], axis=0)


# revision 21
# speedup vs baseline: 1.0320x; 1.0161x over previous
"""Binarized 3x3 conv (BinarizeConv2dSDP) for one TRN2 chip (8 NeuronCores).

Reference computation:
    out = conv2d(sign(x), sign(M), stride=1, pad=1) * Alpha      (all fp32)
    x: (32, 256, 56, 56)   M: (256, 256, 3, 3)   Alpha: (256, 1, 1)

Strategy (per the data-parallel sharding hint):
  - Shard x over batch: 4 images per core; replicate M/Alpha on every core.
  - On-core: binarize x and M to fp8 (+/-1 exactly representable), run the
    conv as 9 shifted DoubleRow matmuls (contraction = 256 channels in one
    pass: 128 partitions x 2 pair-rows) accumulating in PSUM, scale by
    Alpha while evacuating PSUM, DMA out fp32.
  - Activations live in SBUF as zero-padded 58x58 images so every (kh,kw)
    tap of the 3x3 kernel is just a flat column offset; one matmul computes
    an 8-output-row strip (8*58 = 464 psum columns, garbage columns at the
    row seams are simply not copied out).
"""

import numpy as np

import concourse.bacc as bacc
import concourse.bass as bass
import concourse.tile as tile
from concourse import masks, mybir
from concourse.bass_utils import run_bass_kernel_spmd

F32 = mybir.dt.float32
BF16 = mybir.dt.bfloat16
FP8 = mybir.dt.float8e4

# ---- problem geometry (hardcoded; kernel.py must be self-contained) ----
N_CORES = 8
NB = 4          # images per core (32 / 8)
C = 256         # in channels  (2 halves of 128 partitions)
O = 256         # out channels (2 tiles of 128 partitions)
H = W = 56
K = 3
PW = H + 2      # padded row width  (58)
NPIX = PW * PW  # padded image size (3364)
PH = 3376       # padded image allocation (multiple of 16 for DoubleRow step)
RS = 8          # output rows per strip
NSTRIP = H // RS        # 7
NCOL = RS * PW          # 464 psum columns per strip (<= 512 fp32 bank)
NVAL = RS * W           # 448 valid columns per strip


def build_nc(paired: bool = True) -> bass.Bass:
    """Build the SPMD Bass program for one core's shard.

    paired=True : fp8 DoubleRow matmuls (K=256 per instruction, 9 per strip)
    paired=False: plain matmuls (K=128, 18 per strip) - debug fallback

    Schedule shape (engine streams follow trace order):
      sync : alpha, 4x w-DMA, 8x x-DMA, out-DMAs
      PE   : 72 warmup MMs, 18 transposes (ot=0), conv pass ot=0
             (18 transposes for ot=1 slipped in warm, mid-pass), conv ot=1
      ACT  : w-signs (ot=0), image 0/1 signs, w-signs (ot=1)
      DVE  : wz+act memsets, wbuf casts, evacuations, image 2/3 converts
             (deferred between early evacuations)
    """
    act_dt = FP8 if paired else BF16
    nc = bacc.Bacc("TRN2")

    x = nc.declare_dram_parameter("x", [NB, C, H, W], F32, isOutput=False)
    m = nc.declare_dram_parameter("m", [O, C, K, K], F32, isOutput=False)
    alpha = nc.declare_dram_parameter("alpha", [O], F32, isOutput=False)
    out = nc.declare_dram_parameter("out", [NB, O, H, W], F32, isOutput=True)

    with tile.TileContext(nc) as tc:
        with (
            tc.tile_pool(name="consts", bufs=1) as consts,
            tc.tile_pool(name="xsrc", bufs=8) as xsrc_pool,
            tc.tile_pool(name="wsrc", bufs=4) as wsrc_pool,
            tc.tile_pool(name="wsgn", bufs=4) as wsgn_pool,
            tc.tile_pool(name="osb", bufs=3) as osb_pool,
            tc.tile_pool(name="warm", bufs=1, space="PSUM") as warm_pool,
            tc.tile_pool(name="ptr", bufs=2, space="PSUM") as ptr_pool,
            tc.tile_pool(name="pmm", bufs=5, space="PSUM") as pmm_pool,
        ):
            # alpha: two tiny scattered DMAs; trigger them first so they get
            # queue credit before the big transfers monopolize the DGE ring
            alpha_sb = consts.tile([128, 2], F32)
            for ot in range(2):
                nc.sync.dma_start(
                    out=alpha_sb[:, ot : ot + 1],
                    in_=alpha.rearrange("(t o) -> t o", t=2)[ot].rearrange(
                        "(o u) -> o u", u=1
                    ),
                )

            # ---- weight DMAs: one per (ot, half) quarter of M ----
            # wbuf[c2, half, k*256 + ot*128 + o] = sign(M[ot*128+o, half*128+c2, kh, kw])
            wbuf = consts.tile([128, 2, K * K * O], act_dt)
            wprep = []
            for ot in range(2):
                for half in range(2):
                    wsrc = wsrc_pool.tile([128, 128 * K * K], F32)
                    nc.sync.dma_start(
                        out=wsrc[:],
                        in_=m[
                            ot * 128 : (ot + 1) * 128, half * 128 : (half + 1) * 128
                        ].rearrange("o c kh kw -> o (c kh kw)"),
                    )
                    wprep.append((ot, half, wsrc))

            # ---- x DMAs: all 8 half-image loads, each with its own buffer
            # (no slot reuse -> no DMA-waits-on-sign ladder) ----
            xtiles = []
            for n in range(NB):
                for half in range(2):
                    xs = xsrc_pool.tile([128, H * W], F32)
                    nc.sync.dma_start(
                        out=xs[:],
                        in_=x[n, half * 128 : (half + 1) * 128].rearrange(
                            "c h w -> c (h w)"
                        ),
                    )
                    xtiles.append((n, half, xs))

            # ---- PE warm-up: dependency-free matmuls so the HAM clock gate
            # reaches 2.4 GHz before the real matmuls start ----
            wz = consts.tile([128, 128], BF16)
            nc.vector.memset(wz[:], 0)
            pwarm = warm_pool.tile([128, 128], F32)
            for _ in range(72):
                nc.tensor.matmul(pwarm[:], wz[:], wz[:], start=True, stop=True)

            # activation buffer: zero borders (DVE, during the prologue)
            act = consts.tile([128, 2 * NB, PH], act_dt)
            for n in range(NB):
                nc.vector.memset(
                    act[:, 2 * n : 2 * n + 2, :]
                    .rearrange("p a b -> p (a b)")
                    .bitcast(mybir.dt.uint32),
                    0,
                )

            identity = consts.tile([128, 128], BF16)
            masks.make_identity(nc, identity[:])

            def w_unit(ot, half, wsrc):
                """sign + 9 PE transposes + 9 DVE casts for one M quarter."""
                wsgn = wsgn_pool.tile([128, 128 * K * K], BF16)
                nc.scalar.sign(wsgn[:], wsrc[:])
                wsgn_ck = wsgn.rearrange("o (c k) -> o c k", k=K * K)
                for kk in range(K * K):
                    tp = ptr_pool.tile([128, 128], BF16)
                    nc.tensor.transpose(tp[:], wsgn_ck[:, :, kk], identity[:])
                    nc.vector.tensor_copy(
                        wbuf[:, half, kk * O + ot * 128 : kk * O + ot * 128 + 128],
                        tp[:],
                    )

            # ot=0 weight tiles now: these 18 transposes gate the first conv
            for ot, half, wsrc in wprep[:2]:
                w_unit(ot, half, wsrc)

            # ---- activations: zero-padded, binarized ----
            # act[c2, 2*n + half, ph*58 + pw] = sign(x[n, half*128+c2, ph-1, pw-1])
            deferred = []
            for n, half, xs in xtiles:
                dst = act[:, 2 * n + half, : PW * PW].rearrange(
                    "p (h w) -> p h w", w=PW
                )[:, 1 : H + 1, 1 : W + 1]
                src = xs.rearrange("p (h w) -> p h w", w=W)
                if n < 2:
                    # ACT engine: sign(x) -> +/-1
                    nc.scalar.sign(dst, src)
                else:
                    # DVE: (x >= 0) - 0.5 -> +/-0.5 (exact in fp8), paid back
                    # by 2x in this image's output scale. Deferred into the
                    # main loop so they sit AFTER early evacuations on the
                    # DVE FIFO and never block PSUM drainage.
                    def _emit(dst=dst, src=src):
                        nc.vector.tensor_scalar(
                            dst,
                            src,
                            0.0,
                            0.5,
                            mybir.AluOpType.is_ge,
                            mybir.AluOpType.subtract,
                        )

                    deferred.append(_emit)

            # ot=1 weight tiles: emitted mid-pass (groups 8/16) so the PE
            # runs them warm and they stay off the startup critical path
            wunits = [lambda u=u: w_unit(*u) for u in wprep[2:]]

            # ---- main conv loop (ot outer: conv can start after 18
            # transposes + image 0; images stream in during the ot=0 pass) ----
            gidx = 0
            for ot in range(2):
                for n in range(NB):
                    for s in range(NSTRIP):
                        pm = pmm_pool.tile([128, NCOL], F32)
                        mm = 0
                        for kk in range(K * K):
                            kh, kw = divmod(kk, K)
                            base = (RS * s + kh) * PW + kw
                            lhsT2 = wbuf[:, :, kk * O + ot * 128 : kk * O + ot * 128 + 128]
                            rhs2 = act[:, 2 * n : 2 * n + 2, base : base + NCOL]
                            if paired:
                                nc.tensor.matmul(
                                    pm[:],
                                    lhsT2,
                                    rhs2,
                                    start=(mm == 0),
                                    stop=(kk == K * K - 1),
                                    perf_mode=mybir.MatmulPerfMode.DoubleRow,
                                )
                                mm += 1
                            else:
                                for half in range(2):
                                    nc.tensor.matmul(
                                        pm[:],
                                        lhsT2[:, half, :],
                                        rhs2[:, half, :],
                                        start=(mm == 0),
                                        stop=(kk == K * K - 1 and half == 1),
                                    )
                                    mm += 1
                        # evacuate valid columns, scaled by per-channel alpha
                        # (2x extra for images binarized to +/-0.5 on DVE)
                        osb = osb_pool.tile([128, NVAL], F32)
                        if n < 2:
                            nc.vector.tensor_scalar_mul(
                                osb.rearrange("p (r w) -> p r w", w=W),
                                pm.rearrange("p (r w) -> p r w", w=PW)[:, :, :W],
                                alpha_sb[:, ot : ot + 1],
                            )
                        else:
                            nc.vector.tensor_scalar(
                                osb.rearrange("p (r w) -> p r w", w=W),
                                pm.rearrange("p (r w) -> p r w", w=PW)[:, :, :W],
                                alpha_sb[:, ot : ot + 1],
                                2.0,
                                mybir.AluOpType.mult,
                                mybir.AluOpType.mult,
                            )
                        nc.sync.dma_start(
                            out=out[
                                n, ot * 128 : (ot + 1) * 128, RS * s : RS * (s + 1), :
                            ].rearrange("o h w -> o (h w)"),
                            in_=osb[:],
                        )
                        if deferred:
                            deferred.pop(0)()
                        if gidx in (8, 16) and wunits:
                            wunits.pop(0)()
                        gidx += 1
    nc.finalize()
    return nc


_NC_CACHE: dict[bool, bass.Bass] = {}


def get_nc(paired: bool = True) -> bass.Bass:
    if paired not in _NC_CACHE:
        _NC_CACHE[paired] = build_nc(paired)
    return _NC_CACHE[paired]


def kernel(x: np.ndarray, M: np.ndarray, Alpha: np.ndarray) -> np.ndarray:
    """Full (unsharded) inputs in, full output out. Runs on 8 NeuronCores."""
    assert x.shape == (N_CORES * NB, C, H, W), x.shape
    nc = get_nc(paired=True)
    x = np.ascontiguousarray(x, dtype=np.float32)
    M = np.ascontiguousarray(M, dtype=np.float32)
    a = np.ascontiguousarray(Alpha, dtype=np.float32).reshape(O)
    in_maps = [
        {"x": x[i * NB : (i + 1) * NB], "m": M, "alpha": a} for i in range(N_CORES)
    ]
    res = run_bass_kernel_spmd(nc, in_maps, list(range(N_CORES)))
    return np.concatenate([res.results[i]["out"] for i in range(N_CORES)], axis=0)


# revision 23
# speedup vs baseline: 1.0478x; 1.0154x over previous
"""Binarized 3x3 conv (BinarizeConv2dSDP) for one TRN2 chip (8 NeuronCores).

Reference computation:
    out = conv2d(sign(x), sign(M), stride=1, pad=1) * Alpha      (all fp32)
    x: (32, 256, 56, 56)   M: (256, 256, 3, 3)   Alpha: (256, 1, 1)

Strategy (per the data-parallel sharding hint):
  - Shard x over batch: 4 images per core; replicate M/Alpha on every core.
  - On-core: binarize x and M to fp8 (+/-1 exactly representable), run the
    conv as 9 shifted DoubleRow matmuls (contraction = 256 channels in one
    pass: 128 partitions x 2 pair-rows) accumulating in PSUM, scale by
    Alpha while evacuating PSUM, DMA out fp32.
  - Activations live in SBUF as zero-padded 58x58 images so every (kh,kw)
    tap of the 3x3 kernel is just a flat column offset; one matmul computes
    an 8-output-row strip (8*58 = 464 psum columns, garbage columns at the
    row seams are simply not copied out).
"""

import numpy as np

import concourse.bacc as bacc
import concourse.bass as bass
import concourse.tile as tile
from concourse import masks, mybir
from concourse.bass_utils import run_bass_kernel_spmd

F32 = mybir.dt.float32
BF16 = mybir.dt.bfloat16
FP8 = mybir.dt.float8e4

# ---- problem geometry (hardcoded; kernel.py must be self-contained) ----
N_CORES = 8
NB = 4          # images per core (32 / 8)
C = 256         # in channels  (2 halves of 128 partitions)
O = 256         # out channels (2 tiles of 128 partitions)
H = W = 56
K = 3
PW = H + 2      # padded row width  (58)
NPIX = PW * PW  # padded image size (3364)
PH = 3376       # padded image allocation (multiple of 16 for DoubleRow step)
RS = 8          # output rows per strip
NSTRIP = H // RS        # 7
NCOL = RS * PW          # 464 psum columns per strip (<= 512 fp32 bank)
NVAL = RS * W           # 448 valid columns per strip


def build_nc(paired: bool = True) -> bass.Bass:
    """Build the SPMD Bass program for one core's shard.

    paired=True : fp8 DoubleRow matmuls (K=256 per instruction, 9 per strip)
    paired=False: plain matmuls (K=128, 18 per strip) - debug fallback

    Schedule shape (engine streams follow trace order):
      sync : alpha, 4x w-DMA, 8x x-DMA, out-DMAs
      PE   : 72 warmup MMs, 18 transposes (ot=0), conv pass ot=0
             (18 transposes for ot=1 slipped in warm, mid-pass), conv ot=1
      ACT  : w-signs (ot=0), image 0/1 signs, w-signs (ot=1)
      DVE  : wz+act memsets, wbuf casts, evacuations, image 2/3 converts
             (deferred between early evacuations)
    """
    act_dt = FP8 if paired else BF16
    nc = bacc.Bacc("TRN2")

    x = nc.declare_dram_parameter("x", [NB, C, H, W], F32, isOutput=False)
    m = nc.declare_dram_parameter("m", [O, C, K, K], F32, isOutput=False)
    alpha = nc.declare_dram_parameter("alpha", [O], F32, isOutput=False)
    out = nc.declare_dram_parameter("out", [NB, O, H, W], F32, isOutput=True)

    with tile.TileContext(nc) as tc:
        with (
            tc.tile_pool(name="consts", bufs=1) as consts,
            tc.tile_pool(name="xsrc", bufs=8) as xsrc_pool,
            tc.tile_pool(name="wsrc", bufs=4) as wsrc_pool,
            tc.tile_pool(name="wsgn", bufs=4) as wsgn_pool,
            tc.tile_pool(name="osb", bufs=3) as osb_pool,
            tc.tile_pool(name="warm", bufs=1, space="PSUM") as warm_pool,
            tc.tile_pool(name="ptr", bufs=2, space="PSUM") as ptr_pool,
            tc.tile_pool(name="pmm", bufs=5, space="PSUM") as pmm_pool,
        ):
            # alpha: two tiny scattered DMAs; trigger them first so they get
            # queue credit before the big transfers monopolize the DGE ring
            alpha_sb = consts.tile([128, 2], F32)
            for ot in range(2):
                nc.sync.dma_start(
                    out=alpha_sb[:, ot : ot + 1],
                    in_=alpha.rearrange("(t o) -> t o", t=2)[ot].rearrange(
                        "(o u) -> o u", u=1
                    ),
                )

            # ---- weight DMAs: one per (ot, half) quarter of M ----
            # wbuf[c2, half, k*256 + ot*128 + o] = sign(M[ot*128+o, half*128+c2, kh, kw])
            wbuf = consts.tile([128, 2, K * K * O], act_dt)
            wprep = []
            for ot in range(2):
                for half in range(2):
                    wsrc = wsrc_pool.tile([128, 128 * K * K], F32)
                    nc.sync.dma_start(
                        out=wsrc[:],
                        in_=m[
                            ot * 128 : (ot + 1) * 128, half * 128 : (half + 1) * 128
                        ].rearrange("o c kh kw -> o (c kh kw)"),
                    )
                    wprep.append((ot, half, wsrc))

            # ---- x DMAs: all 8 half-image loads, each with its own buffer
            # (no slot reuse -> no DMA-waits-on-sign ladder) ----
            xtiles = []
            for n in range(NB):
                for half in range(2):
                    xs = xsrc_pool.tile([128, H * W], F32)
                    nc.sync.dma_start(
                        out=xs[:],
                        in_=x[n, half * 128 : (half + 1) * 128].rearrange(
                            "c h w -> c (h w)"
                        ),
                    )
                    xtiles.append((n, half, xs))

            # ---- PE warm-up: dependency-free matmuls so the HAM clock gate
            # reaches 2.4 GHz before the real matmuls start ----
            wz = consts.tile([128, 128], BF16)
            nc.vector.memset(wz[:], 0)
            pwarm = warm_pool.tile([128, 128], F32)
            for _ in range(72):
                nc.tensor.matmul(pwarm[:], wz[:], wz[:], start=True, stop=True)

            # activation buffer: zero borders (DVE, during the prologue)
            act = consts.tile([128, 2 * NB, PH], act_dt)
            for n in range(NB):
                nc.vector.memset(
                    act[:, 2 * n : 2 * n + 2, :]
                    .rearrange("p a b -> p (a b)")
                    .bitcast(mybir.dt.uint32),
                    0,
                )

            identity = consts.tile([128, 128], BF16)
            masks.make_identity(nc, identity[:])

            def w_unit(ot, half, wsrc):
                """sign + 9 PE transposes + 9 DVE casts for one M quarter."""
                wsgn = wsgn_pool.tile([128, 128 * K * K], BF16)
                nc.scalar.sign(wsgn[:], wsrc[:])
                wsgn_ck = wsgn.rearrange("o (c k) -> o c k", k=K * K)
                for kk in range(K * K):
                    tp = ptr_pool.tile([128, 128], BF16)
                    nc.tensor.transpose(tp[:], wsgn_ck[:, :, kk], identity[:])
                    nc.vector.tensor_copy(
                        wbuf[:, half, kk * O + ot * 128 : kk * O + ot * 128 + 128],
                        tp[:],
                    )

            # ot=0 weight tiles now: these 18 transposes gate the first conv
            for ot, half, wsrc in wprep[:2]:
                w_unit(ot, half, wsrc)

            # ---- activations: zero-padded, binarized ----
            # act[c2, 2*n + half, ph*58 + pw] = sign(x[n, half*128+c2, ph-1, pw-1])
            for n, half, xs in xtiles:
                dst = act[:, 2 * n + half, : PW * PW].rearrange(
                    "p (h w) -> p h w", w=PW
                )[:, 1 : H + 1, 1 : W + 1]
                nc.scalar.sign(dst, xs.rearrange("p (h w) -> p h w", w=W))

            # ot=1 weight tiles: emitted mid-pass (after image 0's groups) so
            # the PE runs them warm, off the startup critical path
            wunits = [lambda u=u: w_unit(*u) for u in wprep[2:]]

            # ---- main conv loop. Image-major with ot outer within each
            # image: the first conv group needs only 18 transposes + image 0,
            # and image n is not needed until ~20 + 24.7*n us ----
            gidx = 0
            for n in range(NB):
                for ot in range(2):
                    for s in range(NSTRIP):
                        pm = pmm_pool.tile([128, NCOL], F32)
                        mm = 0
                        for kk in range(K * K):
                            kh, kw = divmod(kk, K)
                            base = (RS * s + kh) * PW + kw
                            lhsT2 = wbuf[:, :, kk * O + ot * 128 : kk * O + ot * 128 + 128]
                            rhs2 = act[:, 2 * n : 2 * n + 2, base : base + NCOL]
                            if paired:
                                nc.tensor.matmul(
                                    pm[:],
                                    lhsT2,
                                    rhs2,
                                    start=(mm == 0),
                                    stop=(kk == K * K - 1),
                                    perf_mode=mybir.MatmulPerfMode.DoubleRow,
                                )
                                mm += 1
                            else:
                                for half in range(2):
                                    nc.tensor.matmul(
                                        pm[:],
                                        lhsT2[:, half, :],
                                        rhs2[:, half, :],
                                        start=(mm == 0),
                                        stop=(kk == K * K - 1 and half == 1),
                                    )
                                    mm += 1
                        # evacuate valid columns, scaled by per-channel alpha
                        # (2x extra for images binarized to +/-0.5 on DVE)
                        osb = osb_pool.tile([128, NVAL], F32)
                        nc.vector.tensor_scalar_mul(
                            osb.rearrange("p (r w) -> p r w", w=W),
                            pm.rearrange("p (r w) -> p r w", w=PW)[:, :, :W],
                            alpha_sb[:, ot : ot + 1],
                        )
                        nc.sync.dma_start(
                            out=out[
                                n, ot * 128 : (ot + 1) * 128, RS * s : RS * (s + 1), :
                            ].rearrange("o h w -> o (h w)"),
                            in_=osb[:],
                        )
                        if gidx in (2, 4) and wunits:
                            wunits.pop(0)()
                        gidx += 1
    nc.finalize()
    return nc


_NC_CACHE: dict[bool, bass.Bass] = {}


def get_nc(paired: bool = True) -> bass.Bass:
    if paired not in _NC_CACHE:
        _NC_CACHE[paired] = build_nc(paired)
    return _NC_CACHE[paired]


def kernel(x: np.ndarray, M: np.ndarray, Alpha: np.ndarray) -> np.ndarray:
    """Full (unsharded) inputs in, full output out. Runs on 8 NeuronCores."""
    assert x.shape == (N_CORES * NB, C, H, W), x.shape
    nc = get_nc(paired=True)
    x = np.ascontiguousarray(x, dtype=np.float32)
    M = np.ascontiguousarray(M, dtype=np.float32)
    a = np.ascontiguousarray(Alpha, dtype=np.float32).reshape(O)
    in_maps = [
        {"x": x[i * NB : (i + 1) * NB], "m": M, "alpha": a} for i in range(N_CORES)
    ]
    res = run_bass_kernel_spmd(nc, in_maps, list(range(N_CORES)))
    return np.concatenate([res.results[i]["out"] for i in range(N_CORES)], axis=0)


# revision 24
# speedup vs baseline: 1.1034x; 1.0530x over previous
"""Binarized 3x3 conv (BinarizeConv2dSDP) for one TRN2 chip (8 NeuronCores).

Reference computation:
    out = conv2d(sign(x), sign(M), stride=1, pad=1) * Alpha      (all fp32)
    x: (32, 256, 56, 56)   M: (256, 256, 3, 3)   Alpha: (256, 1, 1)

Strategy (per the data-parallel sharding hint):
  - Shard x over batch: 4 images per core; replicate M/Alpha on every core.
  - On-core: binarize x and M to fp8 (+/-1 exactly representable), run the
    conv as 9 shifted DoubleRow matmuls (contraction = 256 channels in one
    pass: 128 partitions x 2 pair-rows) accumulating in PSUM, scale by
    Alpha while evacuating PSUM, DMA out fp32.
  - Activations live in SBUF as zero-padded 58x58 images so every (kh,kw)
    tap of the 3x3 kernel is just a flat column offset; one matmul computes
    an 8-output-row strip (8*58 = 464 psum columns, garbage columns at the
    row seams are simply not copied out).
"""

import numpy as np

import concourse.bacc as bacc
import concourse.bass as bass
import concourse.tile as tile
from concourse import masks, mybir
from concourse.bass_utils import run_bass_kernel_spmd

F32 = mybir.dt.float32
BF16 = mybir.dt.bfloat16
FP8 = mybir.dt.float8e4

# ---- problem geometry (hardcoded; kernel.py must be self-contained) ----
N_CORES = 8
NB = 4          # images per core (32 / 8)
C = 256         # in channels  (2 halves of 128 partitions)
O = 256         # out channels (2 tiles of 128 partitions)
H = W = 56
K = 3
PW = H + 2      # padded row width  (58)
NPIX = PW * PW  # padded image size (3364)
PH = 3376       # padded image allocation (multiple of 16 for DoubleRow step)
RS = 8          # output rows per strip
NSTRIP = H // RS        # 7
NCOL = RS * PW          # 464 psum columns per strip (<= 512 fp32 bank)
NVAL = RS * W           # 448 valid columns per strip


def build_nc(paired: bool = True) -> bass.Bass:
    """Build the SPMD Bass program for one core's shard.

    paired=True : fp8 DoubleRow matmuls (K=256 per instruction, 9 per strip)
    paired=False: plain matmuls (K=128, 18 per strip) - debug fallback

    Schedule shape (engine streams follow trace order):
      sync : alpha, 4x w-DMA, 8x x-DMA, out-DMAs
      PE   : 72 warmup MMs, 18 transposes (ot=0), conv pass ot=0
             (18 transposes for ot=1 slipped in warm, mid-pass), conv ot=1
      ACT  : w-signs (ot=0), image 0/1 signs, w-signs (ot=1)
      DVE  : wz+act memsets, wbuf casts, evacuations, image 2/3 converts
             (deferred between early evacuations)
    """
    act_dt = FP8 if paired else BF16
    nc = bacc.Bacc("TRN2")

    x = nc.declare_dram_parameter("x", [NB, C, H, W], F32, isOutput=False)
    m = nc.declare_dram_parameter("m", [O, C, K, K], F32, isOutput=False)
    alpha = nc.declare_dram_parameter("alpha", [O], F32, isOutput=False)
    out = nc.declare_dram_parameter("out", [NB, O, H, W], F32, isOutput=True)

    with tile.TileContext(nc) as tc:
        with (
            tc.tile_pool(name="consts", bufs=1) as consts,
            tc.tile_pool(name="xsrc", bufs=8) as xsrc_pool,
            tc.tile_pool(name="wsrc", bufs=4) as wsrc_pool,
            tc.tile_pool(name="wsgn", bufs=4) as wsgn_pool,
            tc.tile_pool(name="osb", bufs=6) as osb_pool,
            tc.tile_pool(name="warm", bufs=1, space="PSUM") as warm_pool,
            tc.tile_pool(name="ptr", bufs=2, space="PSUM") as ptr_pool,
            tc.tile_pool(name="pmm", bufs=5, space="PSUM") as pmm_pool,
        ):
            # alpha: two tiny scattered DMAs; trigger them first so they get
            # queue credit before the big transfers monopolize the DGE ring
            alpha_sb = consts.tile([128, 2], F32)
            for ot in range(2):
                nc.sync.dma_start(
                    out=alpha_sb[:, ot : ot + 1],
                    in_=alpha.rearrange("(t o) -> t o", t=2)[ot].rearrange(
                        "(o u) -> o u", u=1
                    ),
                )

            # ---- weight DMAs: one per (ot, half) quarter of M ----
            # wbuf[c2, half, k*256 + ot*128 + o] = sign(M[ot*128+o, half*128+c2, kh, kw])
            wbuf = consts.tile([128, 2, K * K * O], act_dt)
            wprep = []
            for ot in range(2):
                for half in range(2):
                    wsrc = wsrc_pool.tile([128, 128 * K * K], F32)
                    nc.sync.dma_start(
                        out=wsrc[:],
                        in_=m[
                            ot * 128 : (ot + 1) * 128, half * 128 : (half + 1) * 128
                        ].rearrange("o c kh kw -> o (c kh kw)"),
                    )
                    wprep.append((ot, half, wsrc))

            # ---- x DMAs: all 8 half-image loads, each with its own buffer
            # (no slot reuse -> no DMA-waits-on-sign ladder) ----
            xtiles = []
            for n in range(NB):
                for half in range(2):
                    xs = xsrc_pool.tile([128, H * W], F32)
                    nc.sync.dma_start(
                        out=xs[:],
                        in_=x[n, half * 128 : (half + 1) * 128].rearrange(
                            "c h w -> c (h w)"
                        ),
                    )
                    xtiles.append((n, half, xs))

            # ---- PE warm-up: dependency-free matmuls so the HAM clock gate
            # reaches 2.4 GHz before the real matmuls start ----
            wz = consts.tile([128, 128], BF16)
            nc.vector.memset(wz[:], 0)
            pwarm = warm_pool.tile([128, 128], F32)
            for _ in range(72):
                nc.tensor.matmul(pwarm[:], wz[:], wz[:], start=True, stop=True)

            # activation buffer: zero borders (DVE, during the prologue)
            act = consts.tile([128, 2 * NB, PH], act_dt)
            for n in range(NB):
                nc.vector.memset(
                    act[:, 2 * n : 2 * n + 2, :]
                    .rearrange("p a b -> p (a b)")
                    .bitcast(mybir.dt.uint32),
                    0,
                )

            identity = consts.tile([128, 128], BF16)
            masks.make_identity(nc, identity[:])

            def w_unit(ot, half, wsrc):
                """sign + 9 PE transposes + 9 DVE casts for one M quarter."""
                wsgn = wsgn_pool.tile([128, 128 * K * K], BF16)
                nc.scalar.sign(wsgn[:], wsrc[:])
                wsgn_ck = wsgn.rearrange("o (c k) -> o c k", k=K * K)
                for kk in range(K * K):
                    tp = ptr_pool.tile([128, 128], BF16)
                    nc.tensor.transpose(tp[:], wsgn_ck[:, :, kk], identity[:])
                    nc.vector.tensor_copy(
                        wbuf[:, half, kk * O + ot * 128 : kk * O + ot * 128 + 128],
                        tp[:],
                    )

            # ot=0 weight tiles now: these 18 transposes gate the first conv
            for ot, half, wsrc in wprep[:2]:
                w_unit(ot, half, wsrc)

            # ---- activations: zero-padded, binarized ----
            # act[c2, 2*n + half, ph*58 + pw] = sign(x[n, half*128+c2, ph-1, pw-1])
            for n, half, xs in xtiles:
                dst = act[:, 2 * n + half, : PW * PW].rearrange(
                    "p (h w) -> p h w", w=PW
                )[:, 1 : H + 1, 1 : W + 1]
                nc.scalar.sign(dst, xs.rearrange("p (h w) -> p h w", w=W))

            # ot=1 weight tiles: emitted mid-pass (after image 0's groups) so
            # the PE runs them warm, off the startup critical path
            wunits = [lambda u=u: w_unit(*u) for u in wprep[2:]]

            # ---- main conv loop. Image-major with ot outer within each
            # image: the first conv group needs only 18 transposes + image 0,
            # and image n is not needed until ~20 + 24.7*n us ----
            gidx = 0
            for n in range(NB):
                for ot in range(2):
                    for s in range(NSTRIP):
                        pm = pmm_pool.tile([128, NCOL], F32)
                        mm = 0
                        for kk in range(K * K):
                            kh, kw = divmod(kk, K)
                            base = (RS * s + kh) * PW + kw
                            lhsT2 = wbuf[:, :, kk * O + ot * 128 : kk * O + ot * 128 + 128]
                            rhs2 = act[:, 2 * n : 2 * n + 2, base : base + NCOL]
                            if paired:
                                nc.tensor.matmul(
                                    pm[:],
                                    lhsT2,
                                    rhs2,
                                    start=(mm == 0),
                                    stop=(kk == K * K - 1),
                                    perf_mode=mybir.MatmulPerfMode.DoubleRow,
                                )
                                mm += 1
                            else:
                                for half in range(2):
                                    nc.tensor.matmul(
                                        pm[:],
                                        lhsT2[:, half, :],
                                        rhs2[:, half, :],
                                        start=(mm == 0),
                                        stop=(kk == K * K - 1 and half == 1),
                                    )
                                    mm += 1
                        # evacuate valid columns, scaled by per-channel alpha
                        # (2x extra for images binarized to +/-0.5 on DVE)
                        osb = osb_pool.tile([128, NVAL], F32)
                        nc.vector.tensor_scalar_mul(
                            osb.rearrange("p (r w) -> p r w", w=W),
                            pm.rearrange("p (r w) -> p r w", w=PW)[:, :, :W],
                            alpha_sb[:, ot : ot + 1],
                        )
                        # out-DMAs go through GpSimd's DGE ring: the sync
                        # HWDGE ring is saturated by the big input transfers
                        # early on, and waiting for ring credit there stalls
                        # the osb->evac->PSUM->PE chain
                        nc.gpsimd.dma_start(
                            out=out[
                                n, ot * 128 : (ot + 1) * 128, RS * s : RS * (s + 1), :
                            ].rearrange("o h w -> o (h w)"),
                            in_=osb[:],
                        )
                        if gidx in (2, 4) and wunits:
                            wunits.pop(0)()
                        gidx += 1
    nc.finalize()
    return nc


_NC_CACHE: dict[bool, bass.Bass] = {}


def get_nc(paired: bool = True) -> bass.Bass:
    if paired not in _NC_CACHE:
        _NC_CACHE[paired] = build_nc(paired)
    return _NC_CACHE[paired]


def kernel(x: np.ndarray, M: np.ndarray, Alpha: np.ndarray) -> np.ndarray:
    """Full (unsharded) inputs in, full output out. Runs on 8 NeuronCores."""
    assert x.shape == (N_CORES * NB, C, H, W), x.shape
    nc = get_nc(paired=True)
    x = np.ascontiguousarray(x, dtype=np.float32)
    M = np.ascontiguousarray(M, dtype=np.float32)
    a = np.ascontiguousarray(Alpha, dtype=np.float32).reshape(O)
    in_maps = [
        {"x": x[i * NB : (i + 1) * NB], "m": M, "alpha": a} for i in range(N_CORES)
    ]
    res = run_bass_kernel_spmd(nc, in_maps, list(range(N_CORES)))
    return np.concatenate([res.results[i]["out"] for i in range(N_CORES)], axis=0)


# revision 25
# speedup vs baseline: 1.1094x; 1.0055x over previous
"""Binarized 3x3 conv (BinarizeConv2dSDP) for one TRN2 chip (8 NeuronCores).

Reference computation:
    out = conv2d(sign(x), sign(M), stride=1, pad=1) * Alpha      (all fp32)
    x: (32, 256, 56, 56)   M: (256, 256, 3, 3)   Alpha: (256, 1, 1)

Strategy (per the data-parallel sharding hint):
  - Shard x over batch: 4 images per core; replicate M/Alpha on every core.
  - On-core: binarize x and M to fp8 (+/-1 exactly representable), run the
    conv as 9 shifted DoubleRow matmuls (contraction = 256 channels in one
    pass: 128 partitions x 2 pair-rows) accumulating in PSUM, scale by
    Alpha while evacuating PSUM, DMA out fp32.
  - Activations live in SBUF as zero-padded 58x58 images so every (kh,kw)
    tap of the 3x3 kernel is just a flat column offset; one matmul computes
    an 8-output-row strip (8*58 = 464 psum columns, garbage columns at the
    row seams are simply not copied out).
"""

import numpy as np

import concourse.bacc as bacc
import concourse.bass as bass
import concourse.tile as tile
from concourse import masks, mybir
from concourse.bass_utils import run_bass_kernel_spmd

F32 = mybir.dt.float32
BF16 = mybir.dt.bfloat16
FP8 = mybir.dt.float8e4

# ---- problem geometry (hardcoded; kernel.py must be self-contained) ----
N_CORES = 8
NB = 4          # images per core (32 / 8)
C = 256         # in channels  (2 halves of 128 partitions)
O = 256         # out channels (2 tiles of 128 partitions)
H = W = 56
K = 3
PW = H + 2      # padded row width  (58)
NPIX = PW * PW  # padded image size (3364)
PH = 3376       # padded image allocation (multiple of 16 for DoubleRow step)
RS = 8          # output rows per strip
NSTRIP = H // RS        # 7
NCOL = RS * PW          # 464 psum columns per strip (<= 512 fp32 bank)
NVAL = RS * W           # 448 valid columns per strip


def build_nc(paired: bool = True) -> bass.Bass:
    """Build the SPMD Bass program for one core's shard.

    paired=True : fp8 DoubleRow matmuls (K=256 per instruction, 9 per strip)
    paired=False: plain matmuls (K=128, 18 per strip) - debug fallback

    Schedule shape (engine streams follow trace order):
      sync : alpha, 4x w-DMA, 8x x-DMA, out-DMAs
      PE   : 72 warmup MMs, 18 transposes (ot=0), conv pass ot=0
             (18 transposes for ot=1 slipped in warm, mid-pass), conv ot=1
      ACT  : w-signs (ot=0), image 0/1 signs, w-signs (ot=1)
      DVE  : wz+act memsets, wbuf casts, evacuations, image 2/3 converts
             (deferred between early evacuations)
    """
    act_dt = FP8 if paired else BF16
    nc = bacc.Bacc("TRN2")

    x = nc.declare_dram_parameter("x", [NB, C, H, W], F32, isOutput=False)
    m = nc.declare_dram_parameter("m", [O, C, K, K], F32, isOutput=False)
    alpha = nc.declare_dram_parameter("alpha", [O], F32, isOutput=False)
    out = nc.declare_dram_parameter("out", [NB, O, H, W], F32, isOutput=True)

    with tile.TileContext(nc) as tc:
        with (
            tc.tile_pool(name="consts", bufs=1) as consts,
            tc.tile_pool(name="xsrc", bufs=8) as xsrc_pool,
            tc.tile_pool(name="wsrc", bufs=4) as wsrc_pool,
            tc.tile_pool(name="wsgn", bufs=4) as wsgn_pool,
            tc.tile_pool(name="osb", bufs=6) as osb_pool,
            tc.tile_pool(name="warm", bufs=1, space="PSUM") as warm_pool,
            tc.tile_pool(name="ptr", bufs=2, space="PSUM") as ptr_pool,
            tc.tile_pool(name="pmm", bufs=5, space="PSUM") as pmm_pool,
        ):
            # alpha: two tiny scattered DMAs; trigger them first so they get
            # queue credit before the big transfers monopolize the DGE ring
            alpha_sb = consts.tile([128, 2], F32)
            for ot in range(2):
                nc.sync.dma_start(
                    out=alpha_sb[:, ot : ot + 1],
                    in_=alpha.rearrange("(t o) -> t o", t=2)[ot].rearrange(
                        "(o u) -> o u", u=1
                    ),
                )

            # ---- weight DMAs: one per (ot, half) quarter of M ----
            # wbuf[c2, half, k*256 + ot*128 + o] = sign(M[ot*128+o, half*128+c2, kh, kw])
            wbuf = consts.tile([128, 2, K * K * O], act_dt)

            def w_dma(ot, half):
                wsrc = wsrc_pool.tile([128, 128 * K * K], F32)
                nc.sync.dma_start(
                    out=wsrc[:],
                    in_=m[
                        ot * 128 : (ot + 1) * 128, half * 128 : (half + 1) * 128
                    ].rearrange("o c kh kw -> o (c kh kw)"),
                )
                return (ot, half, wsrc)

            def x_dma(n, half):
                xs = xsrc_pool.tile([128, H * W], F32)
                nc.sync.dma_start(
                    out=xs[:],
                    in_=x[n, half * 128 : (half + 1) * 128].rearrange(
                        "c h w -> c (h w)"
                    ),
                )
                return (n, half, xs)

            # DGE drains transfers roughly in issue order at full aggregate
            # bandwidth, so issue order = criticality: weights for the ot=0
            # transposes, then image 0, then the rest. Each x load has its
            # own buffer (no slot reuse -> no DMA-waits-on-sign ladder).
            wprep = [w_dma(0, 0), w_dma(0, 1)]
            xtiles = [x_dma(0, 0), x_dma(0, 1)]
            wprep += [w_dma(1, 0), w_dma(1, 1)]
            for n in range(1, NB):
                for half in range(2):
                    xtiles.append(x_dma(n, half))

            # ---- PE warm-up: dependency-free matmuls so the HAM clock gate
            # reaches 2.4 GHz before the real matmuls start ----
            wz = consts.tile([128, 128], BF16)
            nc.vector.memset(wz[:], 0)
            pwarm = warm_pool.tile([128, 128], F32)
            for _ in range(72):
                nc.tensor.matmul(pwarm[:], wz[:], wz[:], start=True, stop=True)

            # activation buffer: zero borders (DVE, during the prologue)
            act = consts.tile([128, 2 * NB, PH], act_dt)
            for n in range(NB):
                nc.vector.memset(
                    act[:, 2 * n : 2 * n + 2, :]
                    .rearrange("p a b -> p (a b)")
                    .bitcast(mybir.dt.uint32),
                    0,
                )

            identity = consts.tile([128, 128], BF16)
            masks.make_identity(nc, identity[:])

            def w_unit(ot, half, wsrc):
                """sign + 9 PE transposes + 9 DVE casts for one M quarter."""
                wsgn = wsgn_pool.tile([128, 128 * K * K], BF16)
                nc.scalar.sign(wsgn[:], wsrc[:])
                wsgn_ck = wsgn.rearrange("o (c k) -> o c k", k=K * K)
                for kk in range(K * K):
                    tp = ptr_pool.tile([128, 128], BF16)
                    nc.tensor.transpose(tp[:], wsgn_ck[:, :, kk], identity[:])
                    nc.vector.tensor_copy(
                        wbuf[:, half, kk * O + ot * 128 : kk * O + ot * 128 + 128],
                        tp[:],
                    )

            # ot=0 weight tiles now: these 18 transposes gate the first conv
            for ot, half, wsrc in wprep[:2]:
                w_unit(ot, half, wsrc)

            # ---- activations: zero-padded, binarized ----
            # act[c2, 2*n + half, ph*58 + pw] = sign(x[n, half*128+c2, ph-1, pw-1])
            for n, half, xs in xtiles:
                dst = act[:, 2 * n + half, : PW * PW].rearrange(
                    "p (h w) -> p h w", w=PW
                )[:, 1 : H + 1, 1 : W + 1]
                nc.scalar.sign(dst, xs.rearrange("p (h w) -> p h w", w=W))

            # ot=1 weight tiles: emitted mid-pass (after image 0's groups) so
            # the PE runs them warm, off the startup critical path
            wunits = [lambda u=u: w_unit(*u) for u in wprep[2:]]

            # ---- main conv loop. Image-major with ot outer within each
            # image: the first conv group needs only 18 transposes + image 0,
            # and image n is not needed until ~20 + 24.7*n us ----
            gidx = 0
            for n in range(NB):
                for ot in range(2):
                    for s in range(NSTRIP):
                        pm = pmm_pool.tile([128, NCOL], F32)
                        mm = 0
                        for kk in range(K * K):
                            kh, kw = divmod(kk, K)
                            base = (RS * s + kh) * PW + kw
                            lhsT2 = wbuf[:, :, kk * O + ot * 128 : kk * O + ot * 128 + 128]
                            rhs2 = act[:, 2 * n : 2 * n + 2, base : base + NCOL]
                            if paired:
                                nc.tensor.matmul(
                                    pm[:],
                                    lhsT2,
                                    rhs2,
                                    start=(mm == 0),
                                    stop=(kk == K * K - 1),
                                    perf_mode=mybir.MatmulPerfMode.DoubleRow,
                                )
                                mm += 1
                            else:
                                for half in range(2):
                                    nc.tensor.matmul(
                                        pm[:],
                                        lhsT2[:, half, :],
                                        rhs2[:, half, :],
                                        start=(mm == 0),
                                        stop=(kk == K * K - 1 and half == 1),
                                    )
                                    mm += 1
                        # evacuate valid columns, scaled by per-channel alpha
                        # (2x extra for images binarized to +/-0.5 on DVE)
                        osb = osb_pool.tile([128, NVAL], F32)
                        nc.vector.tensor_scalar_mul(
                            osb.rearrange("p (r w) -> p r w", w=W),
                            pm.rearrange("p (r w) -> p r w", w=PW)[:, :, :W],
                            alpha_sb[:, ot : ot + 1],
                        )
                        # Early out-DMAs go through GpSimd's DGE ring: the
                        # sync HWDGE ring is saturated by the big input
                        # transfers at first, and waiting for ring credit
                        # there stalls the osb->evac->PSUM->PE chain. Late
                        # out-DMAs return to the (faster) HWDGE ring, which
                        # is idle once the inputs are in - the SWDGE ring is
                        # slow to drain the final transfers.
                        eng = nc.gpsimd if gidx < 24 else nc.sync
                        eng.dma_start(
                            out=out[
                                n, ot * 128 : (ot + 1) * 128, RS * s : RS * (s + 1), :
                            ].rearrange("o h w -> o (h w)"),
                            in_=osb[:],
                        )
                        if gidx in (4, 6) and wunits:
                            wunits.pop(0)()
                        gidx += 1
    nc.finalize()
    return nc


_NC_CACHE: dict[bool, bass.Bass] = {}


def get_nc(paired: bool = True) -> bass.Bass:
    if paired not in _NC_CACHE:
        _NC_CACHE[paired] = build_nc(paired)
    return _NC_CACHE[paired]


def kernel(x: np.ndarray, M: np.ndarray, Alpha: np.ndarray) -> np.ndarray:
    """Full (unsharded) inputs in, full output out. Runs on 8 NeuronCores."""
    assert x.shape == (N_CORES * NB, C, H, W), x.shape
    nc = get_nc(paired=True)
    x = np.ascontiguousarray(x, dtype=np.float32)
    M = np.ascontiguousarray(M, dtype=np.float32)
    a = np.ascontiguousarray(Alpha, dtype=np.float32).reshape(O)
    in_maps = [
        {"x": x[i * NB : (i + 1) * NB], "m": M, "alpha": a} for i in range(N_CORES)
    ]
    res = run_bass_kernel_spmd(nc, in_maps, list(range(N_CORES)))
    return np.concatenate([res.results[i]["out"] for i in range(N_CORES)], axis=0)


# revision 26
# speedup vs baseline: 1.1335x; 1.0217x over previous
"""Binarized 3x3 conv (BinarizeConv2dSDP) for one TRN2 chip (8 NeuronCores).

Reference computation:
    out = conv2d(sign(x), sign(M), stride=1, pad=1) * Alpha      (all fp32)
    x: (32, 256, 56, 56)   M: (256, 256, 3, 3)   Alpha: (256, 1, 1)

Strategy (per the data-parallel sharding hint):
  - Shard x over batch: 4 images per core; replicate M/Alpha on every core.
  - On-core: binarize x and M to fp8 (+/-1 exactly representable), run the
    conv as 9 shifted DoubleRow matmuls (contraction = 256 channels in one
    pass: 128 partitions x 2 pair-rows) accumulating in PSUM, scale by
    Alpha while evacuating PSUM, DMA out fp32.
  - Activations live in SBUF as zero-padded 58x58 images so every (kh,kw)
    tap of the 3x3 kernel is just a flat column offset; one matmul computes
    an 8-output-row strip (8*58 = 464 psum columns, garbage columns at the
    row seams are simply not copied out).
"""

import numpy as np

import concourse.bacc as bacc
import concourse.bass as bass
import concourse.tile as tile
from concourse import masks, mybir
from concourse.bass_utils import run_bass_kernel_spmd

F32 = mybir.dt.float32
BF16 = mybir.dt.bfloat16
FP8 = mybir.dt.float8e4

# ---- problem geometry (hardcoded; kernel.py must be self-contained) ----
N_CORES = 8
NB = 4          # images per core (32 / 8)
C = 256         # in channels  (2 halves of 128 partitions)
O = 256         # out channels (2 tiles of 128 partitions)
H = W = 56
K = 3
PW = H + 2      # padded row width  (58)
NPIX = PW * PW  # padded image size (3364)
PH = 3376       # padded image allocation (multiple of 16 for DoubleRow step)
RS = 8          # output rows per strip
NSTRIP = H // RS        # 7
NCOL = RS * PW          # 464 psum columns per strip (<= 512 fp32 bank)
NVAL = RS * W           # 448 valid columns per strip


def build_nc(paired: bool = True) -> bass.Bass:
    """Build the SPMD Bass program for one core's shard.

    paired=True : fp8 DoubleRow matmuls (K=256 per instruction, 9 per strip)
    paired=False: plain matmuls (K=128, 18 per strip) - debug fallback

    Schedule shape (engine streams follow trace order):
      sync : alpha, 4x w-DMA, 8x x-DMA, out-DMAs
      PE   : 72 warmup MMs, 18 transposes (ot=0), conv pass ot=0
             (18 transposes for ot=1 slipped in warm, mid-pass), conv ot=1
      ACT  : w-signs (ot=0), image 0/1 signs, w-signs (ot=1)
      DVE  : wz+act memsets, wbuf casts, evacuations, image 2/3 converts
             (deferred between early evacuations)
    """
    act_dt = FP8 if paired else BF16
    nc = bacc.Bacc("TRN2")

    x = nc.declare_dram_parameter("x", [NB, C, H, W], F32, isOutput=False)
    m = nc.declare_dram_parameter("m", [O, C, K, K], F32, isOutput=False)
    alpha = nc.declare_dram_parameter("alpha", [O], F32, isOutput=False)
    out = nc.declare_dram_parameter("out", [NB, O, H, W], F32, isOutput=True)

    with tile.TileContext(nc) as tc:
        with (
            tc.tile_pool(name="consts", bufs=1) as consts,
            tc.tile_pool(name="xsrc", bufs=6) as xsrc_pool,
            tc.tile_pool(name="xsrc0", bufs=4) as xsrc0_pool,
            tc.tile_pool(name="wsrc", bufs=4) as wsrc_pool,
            tc.tile_pool(name="wsgn", bufs=4) as wsgn_pool,
            tc.tile_pool(name="osb", bufs=6) as osb_pool,
            tc.tile_pool(name="warm", bufs=1, space="PSUM") as warm_pool,
            tc.tile_pool(name="ptr", bufs=2, space="PSUM") as ptr_pool,
            tc.tile_pool(name="pmm", bufs=5, space="PSUM") as pmm_pool,
        ):
            # alpha: two tiny scattered DMAs; trigger them first so they get
            # queue credit before the big transfers monopolize the DGE ring
            alpha_sb = consts.tile([128, 2], F32)
            for ot in range(2):
                nc.sync.dma_start(
                    out=alpha_sb[:, ot : ot + 1],
                    in_=alpha.rearrange("(t o) -> t o", t=2)[ot].rearrange(
                        "(o u) -> o u", u=1
                    ),
                )

            # ---- weight DMAs: one per (ot, half) quarter of M ----
            # wbuf[c2, half, k*256 + ot*128 + o] = sign(M[ot*128+o, half*128+c2, kh, kw])
            wbuf = consts.tile([128, 2, K * K * O], act_dt)

            def w_dma(ot, half):
                wsrc = wsrc_pool.tile([128, 128 * K * K], F32)
                nc.sync.dma_start(
                    out=wsrc[:],
                    in_=m[
                        ot * 128 : (ot + 1) * 128, half * 128 : (half + 1) * 128
                    ].rearrange("o c kh kw -> o (c kh kw)"),
                )
                return (ot, half, wsrc)

            def x_dma(n, half):
                xs = xsrc_pool.tile([128, H * W], F32)
                nc.sync.dma_start(
                    out=xs[:],
                    in_=x[n, half * 128 : (half + 1) * 128].rearrange(
                        "c h w -> c (h w)"
                    ),
                )
                return (n, half, xs)

            def x_dma_chunk(n, half, r0, nr):
                xs = xsrc0_pool.tile([128, (H // 2) * W], F32)
                nc.sync.dma_start(
                    out=xs[: , : nr * W],
                    in_=x[n, half * 128 : (half + 1) * 128, r0 : r0 + nr, :].rearrange(
                        "c h w -> c (h w)"
                    ),
                )
                return (n, half, r0, nr, xs)

            # DGE drains transfers roughly in issue order at full aggregate
            # bandwidth, so issue order = criticality: weights for the ot=0
            # transposes, then image 0 (its top rows first - strip s only
            # needs rows 8s-1..8s+9), then the rest. Each x load has its own
            # buffer (no slot reuse -> no DMA-waits-on-sign ladder).
            HT = H // 2
            wprep = [w_dma(0, 0), w_dma(0, 1)]
            xchunks = [x_dma_chunk(0, 0, 0, HT), x_dma_chunk(0, 1, 0, HT),
                       x_dma_chunk(0, 0, HT, HT), x_dma_chunk(0, 1, HT, HT)]
            wprep += [w_dma(1, 0), w_dma(1, 1)]
            xtiles = []
            for n in range(1, NB):
                for half in range(2):
                    xtiles.append(x_dma(n, half))

            # ---- PE warm-up: dependency-free matmuls so the HAM clock gate
            # reaches 2.4 GHz before the real matmuls start ----
            wz = consts.tile([128, 128], BF16)
            nc.vector.memset(wz[:], 0)
            pwarm = warm_pool.tile([128, 128], F32)
            for _ in range(72):
                nc.tensor.matmul(pwarm[:], wz[:], wz[:], start=True, stop=True)

            # activation buffer: zero borders (DVE, during the prologue)
            act = consts.tile([128, 2 * NB, PH], act_dt)
            for n in range(NB):
                nc.vector.memset(
                    act[:, 2 * n : 2 * n + 2, :]
                    .rearrange("p a b -> p (a b)")
                    .bitcast(mybir.dt.uint32),
                    0,
                )

            identity = consts.tile([128, 128], BF16)
            masks.make_identity(nc, identity[:])

            def w_unit(ot, half, wsrc):
                """sign + 9 PE transposes + 9 DVE casts for one M quarter."""
                wsgn = wsgn_pool.tile([128, 128 * K * K], BF16)
                nc.scalar.sign(wsgn[:], wsrc[:])
                wsgn_ck = wsgn.rearrange("o (c k) -> o c k", k=K * K)
                for kk in range(K * K):
                    tp = ptr_pool.tile([128, 128], BF16)
                    nc.tensor.transpose(tp[:], wsgn_ck[:, :, kk], identity[:])
                    nc.vector.tensor_copy(
                        wbuf[:, half, kk * O + ot * 128 : kk * O + ot * 128 + 128],
                        tp[:],
                    )

            # ot=0 weight tiles now: these 18 transposes gate the first conv
            for ot, half, wsrc in wprep[:2]:
                w_unit(ot, half, wsrc)

            # ---- activations: zero-padded, binarized ----
            # act[c2, 2*n + half, ph*58 + pw] = sign(x[n, half*128+c2, ph-1, pw-1])
            for n, half, r0, nr, xs in xchunks:
                dst = act[:, 2 * n + half, : PW * PW].rearrange(
                    "p (h w) -> p h w", w=PW
                )[:, 1 + r0 : 1 + r0 + nr, 1 : W + 1]
                nc.scalar.sign(dst, xs[:, : nr * W].rearrange("p (h w) -> p h w", w=W))
            for n, half, xs in xtiles:
                dst = act[:, 2 * n + half, : PW * PW].rearrange(
                    "p (h w) -> p h w", w=PW
                )[:, 1 : H + 1, 1 : W + 1]
                nc.scalar.sign(dst, xs.rearrange("p (h w) -> p h w", w=W))

            # ot=1 weight tiles: emitted mid-pass (after image 0's groups) so
            # the PE runs them warm, off the startup critical path
            wunits = [lambda u=u: w_unit(*u) for u in wprep[2:]]

            # ---- main conv loop. Image-major with ot outer within each
            # image: the first conv group needs only 18 transposes + image 0,
            # and image n is not needed until ~20 + 24.7*n us ----
            gidx = 0
            for n in range(NB):
                for ot in range(2):
                    for s in range(NSTRIP):
                        pm = pmm_pool.tile([128, NCOL], F32)
                        mm = 0
                        for kk in range(K * K):
                            kh, kw = divmod(kk, K)
                            base = (RS * s + kh) * PW + kw
                            lhsT2 = wbuf[:, :, kk * O + ot * 128 : kk * O + ot * 128 + 128]
                            rhs2 = act[:, 2 * n : 2 * n + 2, base : base + NCOL]
                            if paired:
                                nc.tensor.matmul(
                                    pm[:],
                                    lhsT2,
                                    rhs2,
                                    start=(mm == 0),
                                    stop=(kk == K * K - 1),
                                    perf_mode=mybir.MatmulPerfMode.DoubleRow,
                                )
                                mm += 1
                            else:
                                for half in range(2):
                                    nc.tensor.matmul(
                                        pm[:],
                                        lhsT2[:, half, :],
                                        rhs2[:, half, :],
                                        start=(mm == 0),
                                        stop=(kk == K * K - 1 and half == 1),
                                    )
                                    mm += 1
                        # evacuate valid columns, scaled by per-channel alpha
                        # (2x extra for images binarized to +/-0.5 on DVE)
                        osb = osb_pool.tile([128, NVAL], F32)
                        nc.vector.tensor_scalar_mul(
                            osb.rearrange("p (r w) -> p r w", w=W),
                            pm.rearrange("p (r w) -> p r w", w=PW)[:, :, :W],
                            alpha_sb[:, ot : ot + 1],
                        )
                        # Early out-DMAs go through GpSimd's DGE ring: the
                        # sync HWDGE ring is saturated by the big input
                        # transfers at first, and waiting for ring credit
                        # there stalls the osb->evac->PSUM->PE chain. Late
                        # out-DMAs return to the (faster) HWDGE ring, which
                        # is idle once the inputs are in - the SWDGE ring is
                        # slow to drain the final transfers.
                        eng = nc.gpsimd if gidx < 24 else nc.sync
                        eng.dma_start(
                            out=out[
                                n, ot * 128 : (ot + 1) * 128, RS * s : RS * (s + 1), :
                            ].rearrange("o h w -> o (h w)"),
                            in_=osb[:],
                        )
                        if gidx in (4, 6) and wunits:
                            wunits.pop(0)()
                        gidx += 1
    nc.finalize()
    return nc


_NC_CACHE: dict[bool, bass.Bass] = {}


def get_nc(paired: bool = True) -> bass.Bass:
    if paired not in _NC_CACHE:
        _NC_CACHE[paired] = build_nc(paired)
    return _NC_CACHE[paired]


def kernel(x: np.ndarray, M: np.ndarray, Alpha: np.ndarray) -> np.ndarray:
    """Full (unsharded) inputs in, full output out. Runs on 8 NeuronCores."""
    assert x.shape == (N_CORES * NB, C, H, W), x.shape
    nc = get_nc(paired=True)
    x = np.ascontiguousarray(x, dtype=np.float32)
    M = np.ascontiguousarray(M, dtype=np.float32)
    a = np.ascontiguousarray(Alpha, dtype=np.float32).reshape(O)
    in_maps = [
        {"x": x[i * NB : (i + 1) * NB], "m": M, "alpha": a} for i in range(N_CORES)
    ]
    res = run_bass_kernel_spmd(nc, in_maps, list(range(N_CORES)))
    return np.concatenate([res.results[i]["out"] for i in range(N_CORES)], axis=0)


# revision 27
# speedup vs baseline: 1.1354x; 1.0017x over previous
"""Binarized 3x3 conv (BinarizeConv2dSDP) for one TRN2 chip (8 NeuronCores).

Reference computation:
    out = conv2d(sign(x), sign(M), stride=1, pad=1) * Alpha      (all fp32)
    x: (32, 256, 56, 56)   M: (256, 256, 3, 3)   Alpha: (256, 1, 1)

Strategy (per the data-parallel sharding hint):
  - Shard x over batch: 4 images per core; replicate M/Alpha on every core.
  - On-core: binarize x and M to fp8 (+/-1 exactly representable), run the
    conv as 9 shifted DoubleRow matmuls (contraction = 256 channels in one
    pass: 128 partitions x 2 pair-rows) accumulating in PSUM, scale by
    Alpha while evacuating PSUM, DMA out fp32.
  - Activations live in SBUF as zero-padded 58x58 images so every (kh,kw)
    tap of the 3x3 kernel is just a flat column offset; one matmul computes
    an 8-output-row strip (8*58 = 464 psum columns, garbage columns at the
    row seams are simply not copied out).
"""

import numpy as np

import concourse.bacc as bacc
import concourse.bass as bass
import concourse.tile as tile
from concourse import masks, mybir
from concourse.bass_utils import run_bass_kernel_spmd

F32 = mybir.dt.float32
BF16 = mybir.dt.bfloat16
FP8 = mybir.dt.float8e4

# ---- problem geometry (hardcoded; kernel.py must be self-contained) ----
N_CORES = 8
NB = 4          # images per core (32 / 8)
C = 256         # in channels  (2 halves of 128 partitions)
O = 256         # out channels (2 tiles of 128 partitions)
H = W = 56
K = 3
PW = H + 2      # padded row width  (58)
NPIX = PW * PW  # padded image size (3364)
PH = 3376       # padded image allocation (multiple of 16 for DoubleRow step)
RS = 8          # output rows per strip
NSTRIP = H // RS        # 7
NCOL = RS * PW          # 464 psum columns per strip (<= 512 fp32 bank)
NVAL = RS * W           # 448 valid columns per strip


def build_nc(paired: bool = True) -> bass.Bass:
    """Build the SPMD Bass program for one core's shard.

    paired=True : fp8 DoubleRow matmuls (K=256 per instruction, 9 per strip)
    paired=False: plain matmuls (K=128, 18 per strip) - debug fallback

    Schedule shape (engine streams follow trace order):
      sync : alpha, 4x w-DMA, 8x x-DMA, out-DMAs
      PE   : 72 warmup MMs, 18 transposes (ot=0), conv pass ot=0
             (18 transposes for ot=1 slipped in warm, mid-pass), conv ot=1
      ACT  : w-signs (ot=0), image 0/1 signs, w-signs (ot=1)
      DVE  : wz+act memsets, wbuf casts, evacuations, image 2/3 converts
             (deferred between early evacuations)
    """
    act_dt = FP8 if paired else BF16
    nc = bacc.Bacc("TRN2")

    x = nc.declare_dram_parameter("x", [NB, C, H, W], F32, isOutput=False)
    m = nc.declare_dram_parameter("m", [O, C, K, K], F32, isOutput=False)
    alpha = nc.declare_dram_parameter("alpha", [O], F32, isOutput=False)
    out = nc.declare_dram_parameter("out", [NB, O, H, W], F32, isOutput=True)

    with tile.TileContext(nc) as tc:
        with (
            tc.tile_pool(name="consts", bufs=1) as consts,
            tc.tile_pool(name="xsrc", bufs=6) as xsrc_pool,
            tc.tile_pool(name="xsrc0", bufs=4) as xsrc0_pool,
            tc.tile_pool(name="wsrc", bufs=4) as wsrc_pool,
            tc.tile_pool(name="wsgn", bufs=4) as wsgn_pool,
            tc.tile_pool(name="osb", bufs=6) as osb_pool,
            tc.tile_pool(name="ptr", bufs=2, space="PSUM") as ptr_pool,
            tc.tile_pool(name="pmm", bufs=6, space="PSUM") as pmm_pool,
        ):
            # alpha: two tiny scattered DMAs; trigger them first so they get
            # queue credit before the big transfers monopolize the DGE ring
            alpha_sb = consts.tile([128, 2], F32)
            for ot in range(2):
                nc.sync.dma_start(
                    out=alpha_sb[:, ot : ot + 1],
                    in_=alpha.rearrange("(t o) -> t o", t=2)[ot].rearrange(
                        "(o u) -> o u", u=1
                    ),
                )

            # ---- weight DMAs: one per (ot, half) quarter of M ----
            # wbuf[c2, half, k*256 + ot*128 + o] = sign(M[ot*128+o, half*128+c2, kh, kw])
            wbuf = consts.tile([128, 2, K * K * O], act_dt)

            def w_dma(ot, half):
                wsrc = wsrc_pool.tile([128, 128 * K * K], F32)
                nc.sync.dma_start(
                    out=wsrc[:],
                    in_=m[
                        ot * 128 : (ot + 1) * 128, half * 128 : (half + 1) * 128
                    ].rearrange("o c kh kw -> o (c kh kw)"),
                )
                return (ot, half, wsrc)

            def x_dma(n, half):
                xs = xsrc_pool.tile([128, H * W], F32)
                nc.sync.dma_start(
                    out=xs[:],
                    in_=x[n, half * 128 : (half + 1) * 128].rearrange(
                        "c h w -> c (h w)"
                    ),
                )
                return (n, half, xs)

            def x_dma_chunk(n, half, r0, nr):
                xs = xsrc0_pool.tile([128, (H // 2) * W], F32)
                nc.sync.dma_start(
                    out=xs[: , : nr * W],
                    in_=x[n, half * 128 : (half + 1) * 128, r0 : r0 + nr, :].rearrange(
                        "c h w -> c (h w)"
                    ),
                )
                return (n, half, r0, nr, xs)

            # DGE drains transfers roughly in issue order at full aggregate
            # bandwidth, so issue order = criticality: weights for the ot=0
            # transposes, then image 0 (its top rows first - strip s only
            # needs rows 8s-1..8s+9), then the rest. Each x load has its own
            # buffer (no slot reuse -> no DMA-waits-on-sign ladder).
            HT = H // 2
            wprep = [w_dma(0, 0), w_dma(0, 1)]
            xchunks = [x_dma_chunk(0, 0, 0, HT), x_dma_chunk(0, 1, 0, HT),
                       x_dma_chunk(0, 0, HT, HT), x_dma_chunk(0, 1, HT, HT)]
            wprep += [w_dma(1, 0), w_dma(1, 1)]
            xtiles = []
            for n in range(1, NB):
                for half in range(2):
                    xtiles.append(x_dma(n, half))

            # ---- PE warm-up: dependency-free matmuls so the HAM clock gate
            # reaches 2.4 GHz before the real matmuls start ----
            wz = consts.tile([128, 128], BF16)
            nc.vector.memset(wz[:], 0)
            pwarm = pmm_pool.tile([128, NCOL], F32, tag="pm")
            for _ in range(72):
                nc.tensor.matmul(pwarm[:, :128], wz[:], wz[:], start=True, stop=True)

            # activation buffer: zero borders (DVE, during the prologue)
            act = consts.tile([128, 2 * NB, PH], act_dt)
            for n in range(NB):
                nc.vector.memset(
                    act[:, 2 * n : 2 * n + 2, :]
                    .rearrange("p a b -> p (a b)")
                    .bitcast(mybir.dt.uint32),
                    0,
                )

            identity = consts.tile([128, 128], BF16)
            masks.make_identity(nc, identity[:])

            def w_unit(ot, half, wsrc):
                """sign + 9 PE transposes + 9 DVE casts for one M quarter."""
                wsgn = wsgn_pool.tile([128, 128 * K * K], BF16)
                nc.scalar.sign(wsgn[:], wsrc[:])
                wsgn_ck = wsgn.rearrange("o (c k) -> o c k", k=K * K)
                for kk in range(K * K):
                    tp = ptr_pool.tile([128, 128], BF16)
                    nc.tensor.transpose(tp[:], wsgn_ck[:, :, kk], identity[:])
                    nc.vector.tensor_copy(
                        wbuf[:, half, kk * O + ot * 128 : kk * O + ot * 128 + 128],
                        tp[:],
                    )

            # ot=0 weight tiles now: these 18 transposes gate the first conv
            for ot, half, wsrc in wprep[:2]:
                w_unit(ot, half, wsrc)

            # second warm-up burst: fills the PE-idle window while image 0's
            # DMA+sign completes, so conv starts at full clock
            pwarm2 = pmm_pool.tile([128, NCOL], F32, tag="pm")
            for _ in range(45):
                nc.tensor.matmul(pwarm2[:, :128], wz[:], wz[:], start=True, stop=True)

            # ---- activations: zero-padded, binarized ----
            # act[c2, 2*n + half, ph*58 + pw] = sign(x[n, half*128+c2, ph-1, pw-1])
            for n, half, r0, nr, xs in xchunks:
                dst = act[:, 2 * n + half, : PW * PW].rearrange(
                    "p (h w) -> p h w", w=PW
                )[:, 1 + r0 : 1 + r0 + nr, 1 : W + 1]
                nc.scalar.sign(dst, xs[:, : nr * W].rearrange("p (h w) -> p h w", w=W))
            for n, half, xs in xtiles:
                dst = act[:, 2 * n + half, : PW * PW].rearrange(
                    "p (h w) -> p h w", w=PW
                )[:, 1 : H + 1, 1 : W + 1]
                nc.scalar.sign(dst, xs.rearrange("p (h w) -> p h w", w=W))

            # ot=1 weight tiles: emitted mid-pass (after image 0's groups) so
            # the PE runs them warm, off the startup critical path
            wunits = [lambda u=u: w_unit(*u) for u in wprep[2:]]

            # ---- main conv loop. Image-major with ot outer within each
            # image: the first conv group needs only 18 transposes + image 0,
            # and image n is not needed until ~20 + 24.7*n us ----
            gidx = 0
            for n in range(NB):
                for ot in range(2):
                    for s in range(NSTRIP):
                        pm = pmm_pool.tile([128, NCOL], F32)
                        mm = 0
                        for kk in range(K * K):
                            kh, kw = divmod(kk, K)
                            base = (RS * s + kh) * PW + kw
                            lhsT2 = wbuf[:, :, kk * O + ot * 128 : kk * O + ot * 128 + 128]
                            rhs2 = act[:, 2 * n : 2 * n + 2, base : base + NCOL]
                            if paired:
                                nc.tensor.matmul(
                                    pm[:],
                                    lhsT2,
                                    rhs2,
                                    start=(mm == 0),
                                    stop=(kk == K * K - 1),
                                    perf_mode=mybir.MatmulPerfMode.DoubleRow,
                                )
                                mm += 1
                            else:
                                for half in range(2):
                                    nc.tensor.matmul(
                                        pm[:],
                                        lhsT2[:, half, :],
                                        rhs2[:, half, :],
                                        start=(mm == 0),
                                        stop=(kk == K * K - 1 and half == 1),
                                    )
                                    mm += 1
                        # evacuate valid columns, scaled by per-channel alpha
                        # (2x extra for images binarized to +/-0.5 on DVE)
                        osb = osb_pool.tile([128, NVAL], F32)
                        nc.vector.tensor_scalar_mul(
                            osb.rearrange("p (r w) -> p r w", w=W),
                            pm.rearrange("p (r w) -> p r w", w=PW)[:, :, :W],
                            alpha_sb[:, ot : ot + 1],
                        )
                        # Early out-DMAs go through GpSimd's DGE ring: the
                        # sync HWDGE ring is saturated by the big input
                        # transfers at first, and waiting for ring credit
                        # there stalls the osb->evac->PSUM->PE chain. Late
                        # out-DMAs return to the (faster) HWDGE ring, which
                        # is idle once the inputs are in - the SWDGE ring is
                        # slow to drain the final transfers.
                        eng = nc.gpsimd if gidx < 24 else nc.sync
                        eng.dma_start(
                            out=out[
                                n, ot * 128 : (ot + 1) * 128, RS * s : RS * (s + 1), :
                            ].rearrange("o h w -> o (h w)"),
                            in_=osb[:],
                        )
                        if gidx in (4, 6) and wunits:
                            wunits.pop(0)()
                        gidx += 1
    nc.finalize()
    return nc


_NC_CACHE: dict[bool, bass.Bass] = {}


def get_nc(paired: bool = True) -> bass.Bass:
    if paired not in _NC_CACHE:
        _NC_CACHE[paired] = build_nc(paired)
    return _NC_CACHE[paired]


def kernel(x: np.ndarray, M: np.ndarray, Alpha: np.ndarray) -> np.ndarray:
    """Full (unsharded) inputs in, full output out. Runs on 8 NeuronCores."""
    assert x.shape == (N_CORES * NB, C, H, W), x.shape
    nc = get_nc(paired=True)
    x = np.ascontiguousarray(x, dtype=np.float32)
    M = np.ascontiguousarray(M, dtype=np.float32)
    a = np.ascontiguousarray(Alpha, dtype=np.float32).reshape(O)
    in_maps = [
        {"x": x[i * NB : (i + 1) * NB], "m": M, "alpha": a} for i in range(N_CORES)
    ]
    res = run_bass_kernel_spmd(nc, in_maps, list(range(N_CORES)))
    return np.concatenate([res.results[i]["out"] for i in range(N_CORES)], axis=0)


# revision 28
# speedup vs baseline: 1.1470x; 1.0102x over previous
"""Binarized 3x3 conv (BinarizeConv2dSDP) for one TRN2 chip (8 NeuronCores).

Reference computation:
    out = conv2d(sign(x), sign(M), stride=1, pad=1) * Alpha      (all fp32)
    x: (32, 256, 56, 56)   M: (256, 256, 3, 3)   Alpha: (256, 1, 1)

Strategy (per the data-parallel sharding hint):
  - Shard x over batch: 4 images per core; replicate M/Alpha on every core.
  - On-core: binarize x and M to fp8 (+/-1 exactly representable), run the
    conv as 9 shifted DoubleRow matmuls (contraction = 256 channels in one
    pass: 128 partitions x 2 pair-rows) accumulating in PSUM, scale by
    Alpha while evacuating PSUM, DMA out fp32.
  - Activations live in SBUF as zero-padded 58x58 images so every (kh,kw)
    tap of the 3x3 kernel is just a flat column offset; one matmul computes
    an 8-output-row strip (8*58 = 464 psum columns, garbage columns at the
    row seams are simply not copied out).
"""

import numpy as np

import concourse.bacc as bacc
import concourse.bass as bass
import concourse.tile as tile
from concourse import masks, mybir
from concourse.bass_utils import run_bass_kernel_spmd

F32 = mybir.dt.float32
BF16 = mybir.dt.bfloat16
FP8 = mybir.dt.float8e4

# ---- problem geometry (hardcoded; kernel.py must be self-contained) ----
N_CORES = 8
NB = 4          # images per core (32 / 8)
C = 256         # in channels  (2 halves of 128 partitions)
O = 256         # out channels (2 tiles of 128 partitions)
H = W = 56
K = 3
PW = H + 2      # padded row width  (58)
NPIX = PW * PW  # padded image size (3364)
PH = 3376       # padded image allocation (multiple of 16 for DoubleRow step)
RS = 8          # output rows per strip
NSTRIP = H // RS        # 7
NCOL = RS * PW          # 464 psum columns per strip (<= 512 fp32 bank)
NVAL = RS * W           # 448 valid columns per strip


def build_nc(paired: bool = True) -> bass.Bass:
    """Build the SPMD Bass program for one core's shard.

    paired=True : fp8 DoubleRow matmuls (K=256 per instruction, 9 per strip)
    paired=False: plain matmuls (K=128, 18 per strip) - debug fallback

    Schedule shape (engine streams follow trace order):
      sync : alpha, 4x w-DMA, 8x x-DMA, out-DMAs
      PE   : 72 warmup MMs, 18 transposes (ot=0), conv pass ot=0
             (18 transposes for ot=1 slipped in warm, mid-pass), conv ot=1
      ACT  : w-signs (ot=0), image 0/1 signs, w-signs (ot=1)
      DVE  : wz+act memsets, wbuf casts, evacuations, image 2/3 converts
             (deferred between early evacuations)
    """
    act_dt = FP8 if paired else BF16
    nc = bacc.Bacc("TRN2")

    x = nc.declare_dram_parameter("x", [NB, C, H, W], F32, isOutput=False)
    m = nc.declare_dram_parameter("m", [O, C, K, K], F32, isOutput=False)
    alpha = nc.declare_dram_parameter("alpha", [O], F32, isOutput=False)
    out = nc.declare_dram_parameter("out", [NB, O, H, W], F32, isOutput=True)

    with tile.TileContext(nc) as tc:
        with (
            tc.tile_pool(name="consts", bufs=1) as consts,
            tc.tile_pool(name="xsrc", bufs=6) as xsrc_pool,
            tc.tile_pool(name="xsrc0", bufs=4) as xsrc0_pool,
            tc.tile_pool(name="wsrc", bufs=4) as wsrc_pool,
            tc.tile_pool(name="wsgn", bufs=4) as wsgn_pool,
            tc.tile_pool(name="osb", bufs=6) as osb_pool,
            tc.tile_pool(name="ptr", bufs=2, space="PSUM") as ptr_pool,
            tc.tile_pool(name="pmm", bufs=6, space="PSUM") as pmm_pool,
        ):
            # alpha: two tiny scattered DMAs; trigger them first so they get
            # queue credit before the big transfers monopolize the DGE ring
            alpha_sb = consts.tile([128, 2], F32)
            for ot in range(2):
                nc.gpsimd.dma_start(
                    out=alpha_sb[:, ot : ot + 1],
                    in_=alpha.rearrange("(t o) -> t o", t=2)[ot].rearrange(
                        "(o u) -> o u", u=1
                    ),
                )

            # ---- weight DMAs: one per (ot, half) quarter of M ----
            # wbuf[c2, half, k*256 + ot*128 + o] = sign(M[ot*128+o, half*128+c2, kh, kw])
            wbuf = consts.tile([128, 2, K * K * O], act_dt)

            def w_dma(ot, half):
                wsrc = wsrc_pool.tile([128, 128 * K * K], F32)
                nc.sync.dma_start(
                    out=wsrc[:],
                    in_=m[
                        ot * 128 : (ot + 1) * 128, half * 128 : (half + 1) * 128
                    ].rearrange("o c kh kw -> o (c kh kw)"),
                )
                return (ot, half, wsrc)

            def x_dma(n, half):
                xs = xsrc_pool.tile([128, H * W], F32)
                nc.sync.dma_start(
                    out=xs[:],
                    in_=x[n, half * 128 : (half + 1) * 128].rearrange(
                        "c h w -> c (h w)"
                    ),
                )
                return (n, half, xs)

            def x_dma_chunk(n, half, r0, nr):
                xs = xsrc0_pool.tile([128, (H // 2) * W], F32)
                nc.sync.dma_start(
                    out=xs[: , : nr * W],
                    in_=x[n, half * 128 : (half + 1) * 128, r0 : r0 + nr, :].rearrange(
                        "c h w -> c (h w)"
                    ),
                )
                return (n, half, r0, nr, xs)

            # DGE drains transfers roughly in issue order at full aggregate
            # bandwidth, so issue order = criticality: weights for the ot=0
            # transposes, then image 0 (its top rows first - strip s only
            # needs rows 8s-1..8s+9), then the rest. Each x load has its own
            # buffer (no slot reuse -> no DMA-waits-on-sign ladder).
            HT = H // 2
            wprep = [w_dma(0, 0), w_dma(0, 1)]
            xchunks = [x_dma_chunk(0, 0, 0, HT), x_dma_chunk(0, 1, 0, HT),
                       x_dma_chunk(0, 0, HT, HT), x_dma_chunk(0, 1, HT, HT)]
            wprep += [w_dma(1, 0), w_dma(1, 1)]
            xtiles = []
            for n in range(1, NB):
                for half in range(2):
                    xtiles.append(x_dma(n, half))

            # ---- PE warm-up: dependency-free matmuls so the HAM clock gate
            # reaches 2.4 GHz before the real matmuls start ----
            wz = consts.tile([128, 128], BF16)
            nc.vector.memset(wz[:], 0)
            pwarm = pmm_pool.tile([128, NCOL], F32, tag="pm")
            for _ in range(72):
                nc.tensor.matmul(pwarm[:, :128], wz[:], wz[:], start=True, stop=True)

            # activation buffer: zero borders (DVE, during the prologue)
            act = consts.tile([128, 2 * NB, PH], act_dt)
            for n in range(NB):
                nc.vector.memset(
                    act[:, 2 * n : 2 * n + 2, :]
                    .rearrange("p a b -> p (a b)")
                    .bitcast(mybir.dt.uint32),
                    0,
                )

            identity = consts.tile([128, 128], BF16)
            masks.make_identity(nc, identity[:])

            def w_unit(ot, half, wsrc):
                """sign + 9 PE transposes + 9 DVE casts for one M quarter."""
                wsgn = wsgn_pool.tile([128, 128 * K * K], BF16)
                nc.scalar.sign(wsgn[:], wsrc[:])
                wsgn_ck = wsgn.rearrange("o (c k) -> o c k", k=K * K)
                for kk in range(K * K):
                    tp = ptr_pool.tile([128, 128], BF16)
                    nc.tensor.transpose(tp[:], wsgn_ck[:, :, kk], identity[:])
                    nc.vector.tensor_copy(
                        wbuf[:, half, kk * O + ot * 128 : kk * O + ot * 128 + 128],
                        tp[:],
                    )

            # ot=0 weight tiles now: these 18 transposes gate the first conv
            for ot, half, wsrc in wprep[:2]:
                w_unit(ot, half, wsrc)

            # second warm-up burst: fills the PE-idle window while image 0's
            # DMA+sign completes, so conv starts at full clock
            pwarm2 = pmm_pool.tile([128, NCOL], F32, tag="pm")
            for _ in range(64):
                nc.tensor.matmul(pwarm2[:, :128], wz[:], wz[:], start=True, stop=True)

            # ---- activations: zero-padded, binarized ----
            # act[c2, 2*n + half, ph*58 + pw] = sign(x[n, half*128+c2, ph-1, pw-1])
            for n, half, r0, nr, xs in xchunks:
                dst = act[:, 2 * n + half, : PW * PW].rearrange(
                    "p (h w) -> p h w", w=PW
                )[:, 1 + r0 : 1 + r0 + nr, 1 : W + 1]
                nc.scalar.sign(dst, xs[:, : nr * W].rearrange("p (h w) -> p h w", w=W))
            for n, half, xs in xtiles:
                dst = act[:, 2 * n + half, : PW * PW].rearrange(
                    "p (h w) -> p h w", w=PW
                )[:, 1 : H + 1, 1 : W + 1]
                nc.scalar.sign(dst, xs.rearrange("p (h w) -> p h w", w=W))

            # ot=1 weight tiles: emitted mid-pass (after image 0's groups) so
            # the PE runs them warm, off the startup critical path
            wunits = [lambda u=u: w_unit(*u) for u in wprep[2:]]

            # ---- main conv loop. Image-major with ot outer within each
            # image: the first conv group needs only 18 transposes + image 0,
            # and image n is not needed until ~20 + 24.7*n us ----
            gidx = 0
            for n in range(NB):
                for ot in range(2):
                    for s in range(NSTRIP):
                        pm = pmm_pool.tile([128, NCOL], F32)
                        mm = 0
                        for kk in range(K * K):
                            kh, kw = divmod(kk, K)
                            base = (RS * s + kh) * PW + kw
                            lhsT2 = wbuf[:, :, kk * O + ot * 128 : kk * O + ot * 128 + 128]
                            rhs2 = act[:, 2 * n : 2 * n + 2, base : base + NCOL]
                            if paired:
                                nc.tensor.matmul(
                                    pm[:],
                                    lhsT2,
                                    rhs2,
                                    start=(mm == 0),
                                    stop=(kk == K * K - 1),
                                    perf_mode=mybir.MatmulPerfMode.DoubleRow,
                                )
                                mm += 1
                            else:
                                for half in range(2):
                                    nc.tensor.matmul(
                                        pm[:],
                                        lhsT2[:, half, :],
                                        rhs2[:, half, :],
                                        start=(mm == 0),
                                        stop=(kk == K * K - 1 and half == 1),
                                    )
                                    mm += 1
                        # evacuate valid columns, scaled by per-channel alpha
                        # (2x extra for images binarized to +/-0.5 on DVE)
                        osb = osb_pool.tile([128, NVAL], F32)
                        nc.vector.tensor_scalar_mul(
                            osb.rearrange("p (r w) -> p r w", w=W),
                            pm.rearrange("p (r w) -> p r w", w=PW)[:, :, :W],
                            alpha_sb[:, ot : ot + 1],
                        )
                        # Early out-DMAs go through GpSimd's DGE ring: the
                        # sync HWDGE ring is saturated by the big input
                        # transfers at first, and waiting for ring credit
                        # there stalls the osb->evac->PSUM->PE chain. Late
                        # out-DMAs return to the (faster) HWDGE ring, which
                        # is idle once the inputs are in - the SWDGE ring is
                        # slow to drain the final transfers.
                        eng = nc.gpsimd if gidx < 24 else nc.sync
                        eng.dma_start(
                            out=out[
                                n, ot * 128 : (ot + 1) * 128, RS * s : RS * (s + 1), :
                            ].rearrange("o h w -> o (h w)"),
                            in_=osb[:],
                        )
                        if gidx in (4, 6) and wunits:
                            wunits.pop(0)()
                        if gidx == 12:
                            # mini warm-up burst: bridges the short idle while
                            # image 1's sign completes, avoiding a HAM
                            # re-throttle cycle
                            pwarm3 = pmm_pool.tile([128, NCOL], F32, tag="pm")
                            for _ in range(24):
                                nc.tensor.matmul(
                                    pwarm3[:, :128], wz[:], wz[:], start=True, stop=True
                                )
                        gidx += 1
    nc.finalize()
    return nc


_NC_CACHE: dict[bool, bass.Bass] = {}


def get_nc(paired: bool = True) -> bass.Bass:
    if paired not in _NC_CACHE:
        _NC_CACHE[paired] = build_nc(paired)
    return _NC_CACHE[paired]


def kernel(x: np.ndarray, M: np.ndarray, Alpha: np.ndarray) -> np.ndarray:
    """Full (unsharded) inputs in, full output out. Runs on 8 NeuronCores."""
    assert x.shape == (N_CORES * NB, C, H, W), x.shape
    nc = get_nc(paired=True)
    x = np.ascontiguousarray(x, dtype=np.float32)
    M = np.ascontiguousarray(M, dtype=np.float32)
    a = np.ascontiguousarray(Alpha, dtype=np.float32).reshape(O)
    in_maps = [
        {"x": x[i * NB : (i + 1) * NB], "m": M, "alpha": a} for i in range(N_CORES)
    ]
    res = run_bass_kernel_spmd(nc, in_maps, list(range(N_CORES)))
    return np.concatenate([res.results[i]["out"] for i in range(N_CORES)], axis=0)


# revision 29
# speedup vs baseline: 1.1578x; 1.0094x over previous
"""Binarized 3x3 conv (BinarizeConv2dSDP) for one TRN2 chip (8 NeuronCores).

Reference computation:
    out = conv2d(sign(x), sign(M), stride=1, pad=1) * Alpha      (all fp32)
    x: (32, 256, 56, 56)   M: (256, 256, 3, 3)   Alpha: (256, 1, 1)

Strategy (per the data-parallel sharding hint):
  - Shard x over batch: 4 images per core; replicate M/Alpha on every core.
  - On-core: binarize x and M to fp8 (+/-1 exactly representable), run the
    conv as 9 shifted DoubleRow matmuls (contraction = 256 channels in one
    pass: 128 partitions x 2 pair-rows) accumulating in PSUM, scale by
    Alpha while evacuating PSUM, DMA out fp32.
  - Activations live in SBUF as zero-padded 58x58 images so every (kh,kw)
    tap of the 3x3 kernel is just a flat column offset; one matmul computes
    an 8-output-row strip (8*58 = 464 psum columns, garbage columns at the
    row seams are simply not copied out).
"""

import numpy as np

import concourse.bacc as bacc
import concourse.bass as bass
import concourse.tile as tile
from concourse import masks, mybir
from concourse.bass_utils import run_bass_kernel_spmd

F32 = mybir.dt.float32
BF16 = mybir.dt.bfloat16
FP8 = mybir.dt.float8e4

# ---- problem geometry (hardcoded; kernel.py must be self-contained) ----
N_CORES = 8
NB = 4          # images per core (32 / 8)
C = 256         # in channels  (2 halves of 128 partitions)
O = 256         # out channels (2 tiles of 128 partitions)
H = W = 56
K = 3
PW = H + 2      # padded row width  (58)
NPIX = PW * PW  # padded image size (3364)
PH = 3376       # padded image allocation (multiple of 16 for DoubleRow step)
RS = 8          # output rows per strip
NSTRIP = H // RS        # 7
NCOL = RS * PW          # 464 psum columns per strip (<= 512 fp32 bank)
NVAL = RS * W           # 448 valid columns per strip


def build_nc(paired: bool = True) -> bass.Bass:
    """Build the SPMD Bass program for one core's shard.

    paired=True : fp8 DoubleRow matmuls (K=256 per instruction, 9 per strip)
    paired=False: plain matmuls (K=128, 18 per strip) - debug fallback

    Schedule shape (engine streams follow trace order):
      sync : alpha, 4x w-DMA, 8x x-DMA, out-DMAs
      PE   : 72 warmup MMs, 18 transposes (ot=0), conv pass ot=0
             (18 transposes for ot=1 slipped in warm, mid-pass), conv ot=1
      ACT  : w-signs (ot=0), image 0/1 signs, w-signs (ot=1)
      DVE  : wz+act memsets, wbuf casts, evacuations, image 2/3 converts
             (deferred between early evacuations)
    """
    act_dt = FP8 if paired else BF16
    nc = bacc.Bacc("TRN2")

    x = nc.declare_dram_parameter("x", [NB, C, H, W], F32, isOutput=False)
    m = nc.declare_dram_parameter("m", [O, C, K, K], F32, isOutput=False)
    alpha = nc.declare_dram_parameter("alpha", [O], F32, isOutput=False)
    out = nc.declare_dram_parameter("out", [NB, O, H, W], F32, isOutput=True)

    with tile.TileContext(nc) as tc:
        with (
            tc.tile_pool(name="consts", bufs=1) as consts,
            tc.tile_pool(name="xsrc", bufs=6) as xsrc_pool,
            tc.tile_pool(name="xsrc0", bufs=4) as xsrc0_pool,
            tc.tile_pool(name="wsrc", bufs=4) as wsrc_pool,
            tc.tile_pool(name="wsgn", bufs=4) as wsgn_pool,
            tc.tile_pool(name="osb", bufs=6) as osb_pool,
            tc.tile_pool(name="ptr", bufs=2, space="PSUM") as ptr_pool,
            tc.tile_pool(name="pmm", bufs=6, space="PSUM") as pmm_pool,
        ):
            # alpha: two tiny scattered DMAs; trigger them first so they get
            # queue credit before the big transfers monopolize the DGE ring
            alpha_sb = consts.tile([128, 2], F32)
            for ot in range(2):
                nc.gpsimd.dma_start(
                    out=alpha_sb[:, ot : ot + 1],
                    in_=alpha.rearrange("(t o) -> t o", t=2)[ot].rearrange(
                        "(o u) -> o u", u=1
                    ),
                )

            # ---- weight DMAs: one per (ot, half) quarter of M ----
            # wbuf[c2, half, k*256 + ot*128 + o] = sign(M[ot*128+o, half*128+c2, kh, kw])
            wbuf = consts.tile([128, 2, K * K * O], act_dt)

            def w_dma(ot, half):
                wsrc = wsrc_pool.tile([128, 128 * K * K], F32)
                nc.sync.dma_start(
                    out=wsrc[:],
                    in_=m[
                        ot * 128 : (ot + 1) * 128, half * 128 : (half + 1) * 128
                    ].rearrange("o c kh kw -> o (c kh kw)"),
                )
                return (ot, half, wsrc)

            def x_dma(n, half):
                xs = xsrc_pool.tile([128, H * W], F32)
                nc.sync.dma_start(
                    out=xs[:],
                    in_=x[n, half * 128 : (half + 1) * 128].rearrange(
                        "c h w -> c (h w)"
                    ),
                )
                return (n, half, xs)

            def x_dma_chunk(n, half, r0, nr):
                xs = xsrc0_pool.tile([128, (H // 2) * W], F32)
                nc.sync.dma_start(
                    out=xs[: , : nr * W],
                    in_=x[n, half * 128 : (half + 1) * 128, r0 : r0 + nr, :].rearrange(
                        "c h w -> c (h w)"
                    ),
                )
                return (n, half, r0, nr, xs)

            # DGE drains transfers roughly in issue order at full aggregate
            # bandwidth, so issue order = criticality: weights for the ot=0
            # transposes, then image 0 (its top rows first - strip s only
            # needs rows 8s-1..8s+9), then the rest. Each x load has its own
            # buffer (no slot reuse -> no DMA-waits-on-sign ladder).
            HT = H // 2
            wprep = [w_dma(0, 0), w_dma(0, 1)]
            xchunks = [x_dma_chunk(0, 0, 0, HT), x_dma_chunk(0, 1, 0, HT),
                       x_dma_chunk(0, 0, HT, HT), x_dma_chunk(0, 1, HT, HT)]
            wprep += [w_dma(1, 0), w_dma(1, 1)]
            xtiles = []
            for n in range(1, NB):
                for half in range(2):
                    xtiles.append(x_dma(n, half))

            # ---- PE warm-up: dependency-free matmuls so the HAM clock gate
            # reaches 2.4 GHz before the real matmuls start ----
            wz = consts.tile([128, 128], BF16)
            nc.vector.memset(wz[:], 0)
            pwarm = pmm_pool.tile([128, NCOL], F32, tag="pm")
            for _ in range(72):
                nc.tensor.matmul(pwarm[:, :128], wz[:], wz[:], start=True, stop=True)

            # activation buffer: zero borders (DVE, during the prologue)
            act = consts.tile([128, 2 * NB, PH], act_dt)
            for n in range(NB):
                nc.vector.memset(
                    act[:, 2 * n : 2 * n + 2, :]
                    .rearrange("p a b -> p (a b)")
                    .bitcast(mybir.dt.uint32),
                    0,
                )

            identity = consts.tile([128, 128], BF16)
            masks.make_identity(nc, identity[:])

            def w_unit(ot, half, wsrc):
                """sign + 9 PE transposes + 9 DVE casts for one M quarter."""
                wsgn = wsgn_pool.tile([128, 128 * K * K], BF16)
                nc.scalar.sign(wsgn[:], wsrc[:])
                wsgn_ck = wsgn.rearrange("o (c k) -> o c k", k=K * K)
                for kk in range(K * K):
                    tp = ptr_pool.tile([128, 128], BF16)
                    nc.tensor.transpose(tp[:], wsgn_ck[:, :, kk], identity[:])
                    nc.vector.tensor_copy(
                        wbuf[:, half, kk * O + ot * 128 : kk * O + ot * 128 + 128],
                        tp[:],
                    )

            # ot=0 weight tiles now: these 18 transposes gate the first conv
            for ot, half, wsrc in wprep[:2]:
                w_unit(ot, half, wsrc)

            # second warm-up burst: fills the PE-idle window while image 0's
            # DMA+sign completes, so conv starts at full clock
            pwarm2 = pmm_pool.tile([128, NCOL], F32, tag="pm")
            for _ in range(76):
                nc.tensor.matmul(pwarm2[:, :128], wz[:], wz[:], start=True, stop=True)

            # ---- activations: zero-padded, binarized ----
            # act[c2, 2*n + half, ph*58 + pw] = sign(x[n, half*128+c2, ph-1, pw-1])
            for n, half, r0, nr, xs in xchunks:
                dst = act[:, 2 * n + half, : PW * PW].rearrange(
                    "p (h w) -> p h w", w=PW
                )[:, 1 + r0 : 1 + r0 + nr, 1 : W + 1]
                nc.scalar.sign(dst, xs[:, : nr * W].rearrange("p (h w) -> p h w", w=W))
            for n, half, xs in xtiles:
                dst = act[:, 2 * n + half, : PW * PW].rearrange(
                    "p (h w) -> p h w", w=PW
                )[:, 1 : H + 1, 1 : W + 1]
                nc.scalar.sign(dst, xs.rearrange("p (h w) -> p h w", w=W))

            # ot=1 weight tiles: emitted mid-pass (after image 0's groups) so
            # the PE runs them warm, off the startup critical path
            wunits = [lambda u=u: w_unit(*u) for u in wprep[2:]]

            # ---- main conv loop. Image-major with ot outer within each
            # image: the first conv group needs only 18 transposes + image 0,
            # and image n is not needed until ~20 + 24.7*n us ----
            gidx = 0
            for n in range(NB):
                for ot in range(2):
                    for s in range(NSTRIP):
                        pm = pmm_pool.tile([128, NCOL], F32)
                        mm = 0
                        for kk in range(K * K):
                            kh, kw = divmod(kk, K)
                            base = (RS * s + kh) * PW + kw
                            lhsT2 = wbuf[:, :, kk * O + ot * 128 : kk * O + ot * 128 + 128]
                            rhs2 = act[:, 2 * n : 2 * n + 2, base : base + NCOL]
                            if paired:
                                nc.tensor.matmul(
                                    pm[:],
                                    lhsT2,
                                    rhs2,
                                    start=(mm == 0),
                                    stop=(kk == K * K - 1),
                                    perf_mode=mybir.MatmulPerfMode.DoubleRow,
                                )
                                mm += 1
                            else:
                                for half in range(2):
                                    nc.tensor.matmul(
                                        pm[:],
                                        lhsT2[:, half, :],
                                        rhs2[:, half, :],
                                        start=(mm == 0),
                                        stop=(kk == K * K - 1 and half == 1),
                                    )
                                    mm += 1
                        # evacuate valid columns, scaled by per-channel alpha
                        # (2x extra for images binarized to +/-0.5 on DVE)
                        osb = osb_pool.tile([128, NVAL], F32)
                        nc.vector.tensor_scalar_mul(
                            osb.rearrange("p (r w) -> p r w", w=W),
                            pm.rearrange("p (r w) -> p r w", w=PW)[:, :, :W],
                            alpha_sb[:, ot : ot + 1],
                        )
                        # Early out-DMAs go through GpSimd's DGE ring: the
                        # sync HWDGE ring is saturated by the big input
                        # transfers at first, and waiting for ring credit
                        # there stalls the osb->evac->PSUM->PE chain. Late
                        # out-DMAs return to the (faster) HWDGE ring, which
                        # is idle once the inputs are in - the SWDGE ring is
                        # slow to drain the final transfers.
                        eng = nc.gpsimd if gidx < 24 else nc.sync
                        eng.dma_start(
                            out=out[
                                n, ot * 128 : (ot + 1) * 128, RS * s : RS * (s + 1), :
                            ].rearrange("o h w -> o (h w)"),
                            in_=osb[:],
                        )
                        if gidx in (4, 6) and wunits:
                            wunits.pop(0)()
                        if gidx == 13:
                            # mini warm-up burst: bridges the short idle while
                            # image 1's sign completes, avoiding a HAM
                            # re-throttle cycle
                            pwarm3 = pmm_pool.tile([128, NCOL], F32, tag="pm")
                            for _ in range(30):
                                nc.tensor.matmul(
                                    pwarm3[:, :128], wz[:], wz[:], start=True, stop=True
                                )
                        gidx += 1
    nc.finalize()
    return nc


_NC_CACHE: dict[bool, bass.Bass] = {}


def get_nc(paired: bool = True) -> bass.Bass:
    if paired not in _NC_CACHE:
        _NC_CACHE[paired] = build_nc(paired)
    return _NC_CACHE[paired]


def kernel(x: np.ndarray, M: np.ndarray, Alpha: np.ndarray) -> np.ndarray:
    """Full (unsharded) inputs in, full output out. Runs on 8 NeuronCores."""
    assert x.shape == (N_CORES * NB, C, H, W), x.shape
    nc = get_nc(paired=True)
    x = np.ascontiguousarray(x, dtype=np.float32)
    M = np.ascontiguousarray(M, dtype=np.float32)
    a = np.ascontiguousarray(Alpha, dtype=np.float32).reshape(O)
    in_maps = [
        {"x": x[i * NB : (i + 1) * NB], "m": M, "alpha": a} for i in range(N_CORES)
    ]
    res = run_bass_kernel_spmd(nc, in_maps, list(range(N_CORES)))
    return np.concatenate([res.results[i]["out"] for i in range(N_CORES)], axis=0)


# revision 30
# speedup vs baseline: 1.1597x; 1.0017x over previous
"""Binarized 3x3 conv (BinarizeConv2dSDP) for one TRN2 chip (8 NeuronCores).

Reference computation:
    out = conv2d(sign(x), sign(M), stride=1, pad=1) * Alpha      (all fp32)
    x: (32, 256, 56, 56)   M: (256, 256, 3, 3)   Alpha: (256, 1, 1)

Strategy (per the data-parallel sharding hint):
  - Shard x over batch: 4 images per core; replicate M/Alpha on every core.
  - On-core: binarize x and M to fp8 (+/-1 exactly representable), run the
    conv as 9 shifted DoubleRow matmuls (contraction = 256 channels in one
    pass: 128 partitions x 2 pair-rows) accumulating in PSUM, scale by
    Alpha while evacuating PSUM, DMA out fp32.
  - Activations live in SBUF as zero-padded 58x58 images so every (kh,kw)
    tap of the 3x3 kernel is just a flat column offset; one matmul computes
    an 8-output-row strip (8*58 = 464 psum columns, garbage columns at the
    row seams are simply not copied out).
"""

import numpy as np

import concourse.bacc as bacc
import concourse.bass as bass
import concourse.tile as tile
from concourse import masks, mybir
from concourse.bass_utils import run_bass_kernel_spmd

F32 = mybir.dt.float32
BF16 = mybir.dt.bfloat16
FP8 = mybir.dt.float8e4

# ---- problem geometry (hardcoded; kernel.py must be self-contained) ----
N_CORES = 8
NB = 4          # images per core (32 / 8)
C = 256         # in channels  (2 halves of 128 partitions)
O = 256         # out channels (2 tiles of 128 partitions)
H = W = 56
K = 3
PW = H + 2      # padded row width  (58)
NPIX = PW * PW  # padded image size (3364)
PH = 3376       # padded image allocation (multiple of 16 for DoubleRow step)
RS = 8          # output rows per strip
NSTRIP = H // RS        # 7
NCOL = RS * PW          # 464 psum columns per strip (<= 512 fp32 bank)
NVAL = RS * W           # 448 valid columns per strip


def build_nc(paired: bool = True) -> bass.Bass:
    """Build the SPMD Bass program for one core's shard.

    paired=True : fp8 DoubleRow matmuls (K=256 per instruction, 9 per strip)
    paired=False: plain matmuls (K=128, 18 per strip) - debug fallback

    Schedule shape (engine streams follow trace order):
      sync : alpha, 4x w-DMA, 8x x-DMA, out-DMAs
      PE   : 72 warmup MMs, 18 transposes (ot=0), conv pass ot=0
             (18 transposes for ot=1 slipped in warm, mid-pass), conv ot=1
      ACT  : w-signs (ot=0), image 0/1 signs, w-signs (ot=1)
      DVE  : wz+act memsets, wbuf casts, evacuations, image 2/3 converts
             (deferred between early evacuations)
    """
    act_dt = FP8 if paired else BF16
    nc = bacc.Bacc("TRN2")

    x = nc.declare_dram_parameter("x", [NB, C, H, W], F32, isOutput=False)
    m = nc.declare_dram_parameter("m", [O, C, K, K], F32, isOutput=False)
    alpha = nc.declare_dram_parameter("alpha", [O], F32, isOutput=False)
    out = nc.declare_dram_parameter("out", [NB, O, H, W], F32, isOutput=True)

    with tile.TileContext(nc) as tc:
        with (
            tc.tile_pool(name="consts", bufs=1) as consts,
            tc.tile_pool(name="xsrc", bufs=6) as xsrc_pool,
            tc.tile_pool(name="xsrc0", bufs=4) as xsrc0_pool,
            tc.tile_pool(name="wsrc", bufs=4) as wsrc_pool,
            tc.tile_pool(name="wsgn", bufs=4) as wsgn_pool,
            tc.tile_pool(name="osb", bufs=6) as osb_pool,
            tc.tile_pool(name="ptr", bufs=2, space="PSUM") as ptr_pool,
            tc.tile_pool(name="pmm", bufs=6, space="PSUM") as pmm_pool,
        ):
            # alpha: two tiny scattered DMAs; trigger them first so they get
            # queue credit before the big transfers monopolize the DGE ring
            alpha_sb = consts.tile([128, 2], F32)
            for ot in range(2):
                nc.gpsimd.dma_start(
                    out=alpha_sb[:, ot : ot + 1],
                    in_=alpha.rearrange("(t o) -> t o", t=2)[ot].rearrange(
                        "(o u) -> o u", u=1
                    ),
                )

            # ---- weight DMAs: one per (ot, half) quarter of M ----
            # wbuf[c2, half, k*256 + ot*128 + o] = sign(M[ot*128+o, half*128+c2, kh, kw])
            wbuf = consts.tile([128, 2, K * K * O], act_dt)

            def w_dma(ot, half):
                wsrc = wsrc_pool.tile([128, 128 * K * K], F32)
                nc.sync.dma_start(
                    out=wsrc[:],
                    in_=m[
                        ot * 128 : (ot + 1) * 128, half * 128 : (half + 1) * 128
                    ].rearrange("o c kh kw -> o (c kh kw)"),
                )
                return (ot, half, wsrc)

            def x_dma(n, half):
                xs = xsrc_pool.tile([128, H * W], F32)
                nc.sync.dma_start(
                    out=xs[:],
                    in_=x[n, half * 128 : (half + 1) * 128].rearrange(
                        "c h w -> c (h w)"
                    ),
                )
                return (n, half, xs)

            def x_dma_chunk(n, half, r0, nr):
                xs = xsrc0_pool.tile([128, (H // 2) * W], F32)
                nc.sync.dma_start(
                    out=xs[: , : nr * W],
                    in_=x[n, half * 128 : (half + 1) * 128, r0 : r0 + nr, :].rearrange(
                        "c h w -> c (h w)"
                    ),
                )
                return (n, half, r0, nr, xs)

            # DGE drains transfers roughly in issue order at full aggregate
            # bandwidth, so issue order = criticality: weights for the ot=0
            # transposes, then image 0 (its top rows first - strip s only
            # needs rows 8s-1..8s+9), then the rest. Each x load has its own
            # buffer (no slot reuse -> no DMA-waits-on-sign ladder).
            HT = H // 2
            wprep = [w_dma(0, 0), w_dma(0, 1)]
            xchunks = [x_dma_chunk(0, 0, 0, HT), x_dma_chunk(0, 1, 0, HT),
                       x_dma_chunk(0, 0, HT, HT), x_dma_chunk(0, 1, HT, HT)]
            wprep += [w_dma(1, 0), w_dma(1, 1)]
            xtiles = []
            for n in range(1, NB):
                for half in range(2):
                    xtiles.append(x_dma(n, half))

            # ---- PE warm-up: dependency-free matmuls so the HAM clock gate
            # reaches 2.4 GHz before the real matmuls start ----
            wz = consts.tile([128, 128], BF16)
            nc.vector.memset(wz[:], 0)
            pwarm = pmm_pool.tile([128, NCOL], F32, tag="pm")
            for _ in range(72):
                nc.tensor.matmul(pwarm[:, :128], wz[:], wz[:], start=True, stop=True)

            # activation buffer: zero borders (DVE, during the prologue)
            act = consts.tile([128, 2 * NB, PH], act_dt)
            for n in range(NB):
                nc.vector.memset(
                    act[:, 2 * n : 2 * n + 2, :]
                    .rearrange("p a b -> p (a b)")
                    .bitcast(mybir.dt.uint32),
                    0,
                )

            identity = consts.tile([128, 128], BF16)
            masks.make_identity(nc, identity[:])

            def w_unit(ot, half, wsrc):
                """sign + 9 PE transposes + 9 DVE casts for one M quarter."""
                wsgn = wsgn_pool.tile([128, 128 * K * K], BF16)
                nc.scalar.sign(wsgn[:], wsrc[:])
                wsgn_ck = wsgn.rearrange("o (c k) -> o c k", k=K * K)
                for kk in range(K * K):
                    tp = ptr_pool.tile([128, 128], BF16)
                    nc.tensor.transpose(tp[:], wsgn_ck[:, :, kk], identity[:])
                    nc.vector.tensor_copy(
                        wbuf[:, half, kk * O + ot * 128 : kk * O + ot * 128 + 128],
                        tp[:],
                    )

            # ot=0 weight tiles now: these 18 transposes gate the first conv
            for ot, half, wsrc in wprep[:2]:
                w_unit(ot, half, wsrc)

            # second warm-up burst: fills the PE-idle window while image 0's
            # DMA+sign completes, so conv starts at full clock
            pwarm2 = pmm_pool.tile([128, NCOL], F32, tag="pm")
            for _ in range(96):
                nc.tensor.matmul(pwarm2[:, :128], wz[:], wz[:], start=True, stop=True)

            # ---- activations: zero-padded, binarized ----
            # act[c2, 2*n + half, ph*58 + pw] = sign(x[n, half*128+c2, ph-1, pw-1])
            for n, half, r0, nr, xs in xchunks:
                dst = act[:, 2 * n + half, : PW * PW].rearrange(
                    "p (h w) -> p h w", w=PW
                )[:, 1 + r0 : 1 + r0 + nr, 1 : W + 1]
                nc.scalar.sign(dst, xs[:, : nr * W].rearrange("p (h w) -> p h w", w=W))
            for n, half, xs in xtiles:
                dst = act[:, 2 * n + half, : PW * PW].rearrange(
                    "p (h w) -> p h w", w=PW
                )[:, 1 : H + 1, 1 : W + 1]
                nc.scalar.sign(dst, xs.rearrange("p (h w) -> p h w", w=W))

            # ot=1 weight tiles: emitted mid-pass (after image 0's groups) so
            # the PE runs them warm, off the startup critical path
            wunits = [lambda u=u: w_unit(*u) for u in wprep[2:]]

            # ---- main conv loop. Image-major with ot outer within each
            # image: the first conv group needs only 18 transposes + image 0,
            # and image n is not needed until ~20 + 24.7*n us ----
            gidx = 0
            for n in range(NB):
                for ot in range(2):
                    for s in range(NSTRIP):
                        pm = pmm_pool.tile([128, NCOL], F32)
                        mm = 0
                        for kk in range(K * K):
                            kh, kw = divmod(kk, K)
                            base = (RS * s + kh) * PW + kw
                            lhsT2 = wbuf[:, :, kk * O + ot * 128 : kk * O + ot * 128 + 128]
                            rhs2 = act[:, 2 * n : 2 * n + 2, base : base + NCOL]
                            if paired:
                                nc.tensor.matmul(
                                    pm[:],
                                    lhsT2,
                                    rhs2,
                                    start=(mm == 0),
                                    stop=(kk == K * K - 1),
                                    perf_mode=mybir.MatmulPerfMode.DoubleRow,
                                )
                                mm += 1
                            else:
                                for half in range(2):
                                    nc.tensor.matmul(
                                        pm[:],
                                        lhsT2[:, half, :],
                                        rhs2[:, half, :],
                                        start=(mm == 0),
                                        stop=(kk == K * K - 1 and half == 1),
                                    )
                                    mm += 1
                        # evacuate valid columns, scaled by per-channel alpha
                        # (2x extra for images binarized to +/-0.5 on DVE)
                        osb = osb_pool.tile([128, NVAL], F32)
                        nc.vector.tensor_scalar_mul(
                            osb.rearrange("p (r w) -> p r w", w=W),
                            pm.rearrange("p (r w) -> p r w", w=PW)[:, :, :W],
                            alpha_sb[:, ot : ot + 1],
                        )
                        # Early out-DMAs go through GpSimd's DGE ring: the
                        # sync HWDGE ring is saturated by the big input
                        # transfers at first, and waiting for ring credit
                        # there stalls the osb->evac->PSUM->PE chain. Late
                        # out-DMAs return to the (faster) HWDGE ring, which
                        # is idle once the inputs are in - the SWDGE ring is
                        # slow to drain the final transfers.
                        eng = nc.gpsimd if gidx < 24 else nc.sync
                        eng.dma_start(
                            out=out[
                                n, ot * 128 : (ot + 1) * 128, RS * s : RS * (s + 1), :
                            ].rearrange("o h w -> o (h w)"),
                            in_=osb[:],
                        )
                        if gidx in (4, 6) and wunits:
                            wunits.pop(0)()
                        if gidx == 13:
                            # mini warm-up burst: bridges the short idle while
                            # image 1's sign completes, avoiding a HAM
                            # re-throttle cycle
                            pwarm3 = pmm_pool.tile([128, NCOL], F32, tag="pm")
                            for _ in range(30):
                                nc.tensor.matmul(
                                    pwarm3[:, :128], wz[:], wz[:], start=True, stop=True
                                )
                        gidx += 1
    nc.finalize()
    return nc


_NC_CACHE: dict[bool, bass.Bass] = {}


def get_nc(paired: bool = True) -> bass.Bass:
    if paired not in _NC_CACHE:
        _NC_CACHE[paired] = build_nc(paired)
    return _NC_CACHE[paired]


def kernel(x: np.ndarray, M: np.ndarray, Alpha: np.ndarray) -> np.ndarray:
    """Full (unsharded) inputs in, full output out. Runs on 8 NeuronCores."""
    assert x.shape == (N_CORES * NB, C, H, W), x.shape
    nc = get_nc(paired=True)
    x = np.ascontiguousarray(x, dtype=np.float32)
    M = np.ascontiguousarray(M, dtype=np.float32)
    a = np.ascontiguousarray(Alpha, dtype=np.float32).reshape(O)
    in_maps = [
        {"x": x[i * NB : (i + 1) * NB], "m": M, "alpha": a} for i in range(N_CORES)
    ]
    res = run_bass_kernel_spmd(nc, in_maps, list(range(N_CORES)))
    return np.concatenate([res.results[i]["out"] for i in range(N_CORES)], axis=0)
